# revision 1
# baseline (speedup 1.0000x reference)
"""Trainium2 Bass kernel for nn_AttentionBlock (B=4, H=W=64, C=64, GroupNorm(8) +
full spatial self-attention), distributed over 8 NeuronCores.

Sharding: core i handles batch b=i//2 and query-half h=i%2 (2048 of the 4096
spatial positions). Each core computes the full GroupNorm and K/V for its
image (cheap) and attention only for its query half. No collectives.

Device layout: channel-on-partition ("xT") layout, with the two position
halves of an image packed onto partitions [(half, channel)] -> 128 partitions.
Scores are computed transposed (kv positions on partitions) so that
- the softmax denominator is a free by-product of the attn@V matmul
  (all-ones column appended to V), and
- the attn@V contraction needs no transposes at all.
exp() runs on ScalarE reading PSUM and writing SBUF directly; everything is
software-pipelined per kv-chunk-pair slot: scores (PE, two concurrent K=64
row-group matmuls) -> exp (ACT) -> attn@V (PE) with a multi-slot lookahead so
both engines stay saturated.

PE matmuls run in bf16 (fp32/fp32r matmuls leave the PE activity monitor
cold at 1.2 GHz on TRN2); GroupNorm statistics, PSUM accumulation and the
residual path stay fp32.
"""

import sys

sys.path.insert(0, "/opt/trn_rl_repo")

import numpy as np

import concourse.bacc as bacc
import concourse.tile as tile
from concourse import mybir

B, H, W, C = 4, 64, 64, 64
HW = H * W  # 4096
HALF = HW // 2  # 2048
EPS = 1e-5
SCALE = C ** -0.5  # folded into exp()

F32 = mybir.dt.float32
MDT = mybir.dt.bfloat16  # PE matmul operand dtype



def build_nc():
    nc = bacc.Bacc("TRN2", debug=False, num_devices=8)

    # ---- DRAM I/O ----
    xp_d = nc.dram_tensor("xp", [128, HALF], F32, kind="ExternalInput")
    wq_d = nc.dram_tensor("wq", [64, 128], MDT, kind="ExternalInput")
    wk_d = nc.dram_tensor("wk", [128, 128], MDT, kind="ExternalInput")
    wv_d = nc.dram_tensor("wv", [128, 128], MDT, kind="ExternalInput")
    wo_d = nc.dram_tensor("wo", [64, 64], MDT, kind="ExternalInput")
    bq_d = nc.dram_tensor("bq", [1, 128], MDT, kind="ExternalInput")
    bk_d = nc.dram_tensor("bk", [1, 128], MDT, kind="ExternalInput")
    bv_d = nc.dram_tensor("bv", [1, 128], MDT, kind="ExternalInput")
    bo_d = nc.dram_tensor("bo", [64, 1], F32, kind="ExternalInput")
    gam_d = nc.dram_tensor("gam", [128, 1], F32, kind="ExternalInput")
    bet_d = nc.dram_tensor("bet", [128, 1], F32, kind="ExternalInput")
    comb_d = nc.dram_tensor("comb", [128, 128], F32, kind="ExternalInput")
    out_d = nc.dram_tensor("out", [64, HALF], F32, kind="ExternalOutput")

    with tile.TileContext(nc) as tc, \
         tc.tile_pool(name="singles", bufs=1) as singles, \
         tc.tile_pool(name="stats", bufs=1) as stats, \
         tc.tile_pool(name="sc_ps", bufs=1, space="PSUM") as sc_ps, \
         tc.tile_pool(name="pacc_ps", bufs=2, space="PSUM") as pacc_ps, \
         tc.tile_pool(name="work", bufs=2) as work:

        # ---- input DMAs: x on the sync queue, weights on gpsimd ----
        x_sb = singles.tile([128, HALF], F32)
        for r in range(4):
            nc.sync.dma_start(
                x_sb[:, 512 * r : 512 * r + 512],
                xp_d.ap()[:, 512 * r : 512 * r + 512],
            )
        wq_sb = singles.tile([64, 128], MDT)
        nc.gpsimd.dma_start(wq_sb[:], wq_d.ap())
        wk_sb = singles.tile([128, 128], MDT)
        nc.gpsimd.dma_start(wk_sb[:], wk_d.ap())
        bq_sb = singles.tile([1, 128], MDT)
        nc.gpsimd.dma_start(bq_sb[:], bq_d.ap())
        bk_sb = singles.tile([1, 128], MDT)
        nc.gpsimd.dma_start(bk_sb[:], bk_d.ap())
        gam_sb = singles.tile([128, 1], F32)
        nc.gpsimd.dma_start(gam_sb[:], gam_d.ap())
        bet_sb = singles.tile([128, 1], F32)
        nc.gpsimd.dma_start(bet_sb[:], bet_d.ap())
        comb_sb = singles.tile([128, 128], F32)
        nc.gpsimd.dma_start(comb_sb[:], comb_d.ap())
        wv_sb = singles.tile([128, 128], MDT)
        nc.gpsimd.dma_start(wv_sb[:], wv_d.ap())
        wo_sb = singles.tile([64, 64], MDT)
        nc.gpsimd.dma_start(wo_sb[:], wo_d.ap())
        bv_sb = singles.tile([1, 128], MDT)
        nc.gpsimd.dma_start(bv_sb[:], bv_d.ap())
        bo_sb = singles.tile([64, 1], F32)
        nc.gpsimd.dma_start(bo_sb[:], bo_d.ap())

        # ---- big SBUF tensors ----
        xn_r = singles.tile([128, HALF], MDT)
        q_dup = singles.tile([128, HALF], MDT)
        kt_sb = singles.tile([128, HALF], MDT)
        v_all = singles.tile([128, 65 * 32], MDT)
        attnexp = singles.tile([128, 512 * 32], MDT)
        out_sb = singles.tile([64, HALF], F32)
        ones_sb = singles.tile([128, 512], MDT)

        ones32 = singles.tile([65, 64], F32)
        # constants via gpsimd (DVE stays free for the stats chain)
        nc.gpsimd.memset(ones32[:], 1.0)
        nc.gpsimd.memset(ones_sb[:], 1.0)
        v3 = v_all[:].rearrange("p (t e) -> p t e", e=65)
        nc.gpsimd.memset(v3[:, :, 64:65], 1.0)

        # pre-warm the exp ACT table set under the DMA shadow
        scr = stats.tile([128, 1], F32)
        nc.vector.memset(scr[:], 1.0)
        nc.scalar.activation(scr[:], scr[:], mybir.ActivationFunctionType.Exp)

        # ---- GroupNorm stats: bn per partition per 512-slice, then a
        # block-diagonal averaging matmul combines across channels ----
        st6 = stats.tile([128, 4, 6], F32)
        mv4 = stats.tile([128, 4, 2], F32)
        for r in range(4):
            nc.vector.bn_stats(st6[:, r, :], x_sb[:, 512 * r : 512 * r + 512])
            nc.vector.bn_aggr(mv4[:, r, :], st6[:, r, :])
        smat = stats.tile([128, 8], F32)  # cols 0-3 mean, 4-7 E[x^2]
        nc.vector.tensor_copy(smat[:, 0:4], mv4[:, :, 0])
        nc.vector.tensor_mul(smat[:, 4:8], mv4[:, :, 0], mv4[:, :, 0])
        nc.vector.tensor_add(smat[:, 4:8], smat[:, 4:8], mv4[:, :, 1])

        cps = pacc_ps.tile([128, 8], F32, tag="pacc")
        nc.tensor.matmul(cps[:], comb_sb[:], smat[:], start=True, stop=True)
        gstat = stats.tile([128, 8], F32)  # 0-3 mean_g, 4-7 E2_g
        nc.vector.tensor_copy(gstat[:], cps[:])

        # var+eps, then rstd = rsqrt via bit-trick seed + Newton steps (DVE)
        ve = stats.tile([128, 4], F32)
        nc.vector.tensor_mul(ve[:], gstat[:, 0:4], gstat[:, 0:4])
        nc.vector.tensor_scalar(
            out=ve[:], in0=ve[:], scalar1=-1.0, scalar2=EPS,
            op0=mybir.AluOpType.mult, op1=mybir.AluOpType.add,
        )
        nc.vector.tensor_add(ve[:], ve[:], gstat[:, 4:8])
        yi = stats.tile([128, 4], mybir.dt.int32)
        nc.vector.tensor_scalar(
            out=yi[:], in0=ve[:].bitcast(mybir.dt.int32), scalar1=1,
            scalar2=None, op0=mybir.AluOpType.logical_shift_right,
        )
        nc.vector.tensor_scalar(
            out=yi[:], in0=yi[:], scalar1=-1, scalar2=0x5F3759DF,
            op0=mybir.AluOpType.mult, op1=mybir.AluOpType.add,
        )
        rstd = stats.tile([128, 4], F32)
        nc.vector.tensor_copy(rstd[:], yi[:].bitcast(F32))
        vh = stats.tile([128, 4], F32)
        nc.vector.tensor_scalar_mul(vh[:], ve[:], -0.5)
        t_nw = stats.tile([128, 4], F32)
        for _ in range(2):
            nc.vector.tensor_mul(t_nw[:], rstd[:], rstd[:])
            nc.vector.tensor_mul(t_nw[:], t_nw[:], vh[:])
            nc.vector.tensor_scalar(
                out=t_nw[:], in0=t_nw[:], scalar1=1.0, scalar2=1.5,
                op0=mybir.AluOpType.mult, op1=mybir.AluOpType.add,
            )
            nc.vector.tensor_mul(rstd[:], rstd[:], t_nw[:])

        gsc = stats.tile([128, 4], F32)
        nc.vector.tensor_scalar_mul(gsc[:], rstd[:], gam_sb[:])
        gbias = stats.tile([128, 4], F32)
        nc.vector.tensor_mul(gbias[:], gstat[:, 0:4], gsc[:])
        nc.vector.tensor_scalar(
            out=gbias[:], in0=gbias[:], scalar1=-1.0, scalar2=bet_sb[:],
            op0=mybir.AluOpType.mult, op1=mybir.AluOpType.add,
        )
        # xn = x * gsc + gbias: bf16 copy for the matmuls via ScalarE (runs
        # parallel to the DVE fp32 pass used by the residual path)
        for r in range(4):
            sl = slice(512 * r, 512 * r + 512)
            nc.scalar.activation(
                xn_r[:, sl], x_sb[:, sl],
                mybir.ActivationFunctionType.Identity,
                bias=gbias[:, r : r + 1], scale=gsc[:, r : r + 1],
            )
            nc.vector.tensor_scalar(
                out=x_sb[:, sl], in0=x_sb[:, sl],
                scalar1=gsc[:, r : r + 1], scalar2=gbias[:, r : r + 1],
                op0=mybir.AluOpType.mult, op1=mybir.AluOpType.add,
            )

        # ---- emission helpers ----
        def emit_qk_slice(t):
            # q^T duplicated on both partition halves (lhsT = [Wq | Wq]);
            # zero-padded into q0/q1 so scores run K=128. k^T packed by half
            # (lhsT = blockdiag(Wk, Wk)). Copies split across ACT and DVE.
            sl = slice(512 * t, 512 * t + 512)
            ps2 = pacc_ps.tile([128, 512], F32, tag="pacc", name=f"kps{t}")
            nc.tensor.matmul(ps2[:], bk_sb[:], ones_sb[0:1, :], start=True,
                             stop=False)
            nc.tensor.matmul(ps2[:], wk_sb[:], xn_r[:, sl], start=False,
                             stop=True)
            nc.vector.tensor_copy(kt_sb[:, sl], ps2[:])
            ps = pacc_ps.tile([128, 512], F32, tag="pacc", name=f"qps{t}")
            nc.tensor.matmul(ps[:], bq_sb[:], ones_sb[0:1, :], start=True,
                             stop=False)
            nc.tensor.matmul(ps[:], wq_sb[:], xn_r[0:64, sl], start=False,
                             stop=True)
            if t == 0:
                # slice 0 gates tile 0's scores: fastest path is ScalarE
                nc.scalar.copy(q_dup[:, sl], ps[:])
            else:
                # slices 1-3 gate only tiles 1-3: keep them off the exp engine
                nc.vector.tensor_copy(q_dup[:, sl], ps[:])

        def emit_v_pair(u):
            # v position-major, two 128-position chunks per matmul
            sl = slice(128 * u, 128 * u + 128)
            ps = pacc_ps.tile([128, 128], F32, tag="pacc", name=f"vps{u}")
            nc.tensor.matmul(ps[:], ones_sb[0:1, 0:128], bv_sb[:], start=True,
                             stop=False)
            nc.tensor.matmul(ps[:], xn_r[:, sl], wv_sb[:], start=False,
                             stop=True)
            nc.vector.tensor_copy(v_all[:, 65 * u : 65 * u + 64], ps[:, 0:64])
            nc.vector.tensor_copy(
                v_all[:, 65 * (u + 16) : 65 * (u + 16) + 64], ps[:, 64:128]
            )

        def aoff(t):
            # attnexp is pair-major: chunk c at 1024c, chunk c+16 at 1024c+512
            return 1024 * t if t < 16 else 1024 * (t - 16) + 512

        SLOT_PAIRS = [[0], [1, 2], [3], [4, 5], [6], [7, 8], [9], [10, 11],
                      [12], [13, 14], [15]]
        NSLOT = len(SLOT_PAIRS)

        def emit_scores_slot(n, si):
            # 1-pair (2-bank) and 2-pair (4-bank) slots alternate; each pair =
            # two K=64 matmuls issued back-to-back into row groups 0 and 64
            # (they run concurrently on the PE array). Pair-major attnexp
            # keeps every exp write contiguous.
            pairs = SLOT_PAIRS[si]
            qsl = slice(512 * n, 512 * n + 512)
            w = 1024 * len(pairs)
            tag = "scA" if len(pairs) == 2 else "scB"
            ps = sc_ps.tile([128, w], F32, tag=tag, name=f"sc{n}_{si}")
            for i, p in enumerate(pairs):
                ksl = slice(128 * p, 128 * p + 128)
                nc.tensor.matmul(ps[:, 1024 * i : 1024 * i + 512],
                                 kt_sb[0:64, ksl], q_dup[0:64, qsl],
                                 start=True, stop=True)
                nc.tensor.matmul(ps[:, 1024 * i + 512 : 1024 * i + 1024],
                                 kt_sb[64:128, ksl], q_dup[64:128, qsl],
                                 start=True, stop=True)
            nc.scalar.activation(
                attnexp[:, 1024 * pairs[0] : 1024 * pairs[0] + w], ps[:],
                mybir.ActivationFunctionType.Exp, scale=SCALE,
            )

        paccs = {}

        def emit_proj_batch(n, bi):
            # kv chunk pair (bi, bi+16) — matches exp production order
            if n not in paccs:
                paccs[n] = pacc_ps.tile([65, 512], F32, tag="pacc",
                                        name=f"pacc{n}")
            pacc = paccs[n]
            for t in (bi, bi + 16):
                nc.tensor.matmul(
                    pacc[:], v_all[:, 65 * t : 65 * t + 65],
                    attnexp[:, aoff(t) : aoff(t) + 512],
                    start=(t == 0), stop=(t == 31),
                )

        def emit_finish_a(n):
            # free the PSUM accumulator ASAP: unnormalized proj rows (bf16,
            # feeds the out-projection) + raw denominator row
            pacc = paccs[n]
            projn_u = work.tile([64, 512], MDT, tag="projn", name=f"pn{n}")
            nc.vector.tensor_copy(projn_u[:], pacc[0:64, :])
            dn_sb = work.tile([65, 512], F32, tag="dn", name=f"dn{n}")
            nc.vector.tensor_copy(dn_sb[64:65, :], pacc[64:65, :])
            return projn_u, dn_sb

        def emit_finish_b(n, projn_u, dn_sb):
            # PE: broadcast raw denom + out-projection (nothing here waits on
            # a reciprocal). DVE then normalizes and applies bias + residual.
            qsl = slice(512 * n, 512 * n + 512)
            bc_ps = pacc_ps.tile([64, 512], F32, tag="pacc", name=f"bc{n}")
            nc.tensor.matmul(bc_ps[:], ones32[64:65, :], dn_sb[64:65, :],
                             start=True, stop=True)
            fps = pacc_ps.tile([64, 512], F32, tag="pacc", name=f"fps{n}")
            nc.tensor.matmul(fps[:], wo_sb[:], projn_u[:], start=True, stop=True)
            bc_sb = work.tile([64, 512], F32, tag="bc", name=f"bcs{n}")
            nc.vector.tensor_copy(bc_sb[:], bc_ps[:])
            with nc.allow_low_precision(reason="softmax denom reciprocal"):
                nc.vector.reciprocal(bc_sb[:], bc_sb[:])
            xb = work.tile([64, 512], F32, tag="xb", name=f"xb{n}")
            nc.vector.tensor_scalar_add(xb[:], x_sb[0:64, qsl], bo_sb[:])
            mn = work.tile([64, 512], F32, tag="mn", name=f"mn{n}")
            nc.vector.tensor_mul(mn[:], fps[:], bc_sb[:])
            nc.vector.tensor_add(out_sb[:, qsl], mn[:], xb[:])
            nc.sync.dma_start(out_d.ap()[:, qsl], out_sb[:, qsl])

        # ---- software-pipelined attention: 4 tiles x 11 slots (16 kv
        # pairs). attn@V trails exp by a pair lag; the previous tile's spill
        # pairs + finish ride the first slots of the next tile. Tile 0's
        # slots also carry the q/k slices and the v pairs. ----
        emit_qk_slice(0)
        V_SCHED = {0: [0, 1], 1: [2, 3], 2: [4, 5], 3: [6, 7], 4: [8, 9],
                   5: [10, 11], 6: [12, 13], 7: [14, 15]}
        QK_SCHED = {0: 1, 1: 2, 2: 3}

        pend = {}
        LAGS = [5, 5, 5, 2]  # pair lag: deep early, shallow for the last tile
        HEADS = {5: {0: [11, 12], 1: [13, 14], 2: [15]}, 2: {0: [14], 1: [15]}}

        def emit_head(n, si):
            hd = HEADS[LAGS[n - 1]]
            last = max(hd)
            if si in hd:
                for p in hd[si]:
                    emit_proj_batch(n - 1, p)
            elif si == last + 1:
                pend[n - 1] = emit_finish_a(n - 1)
            elif si == last + 2:
                emit_finish_b(n - 1, *pend.pop(n - 1))

        for n in range(4):
            done = 0  # pairs produced so far this tile
            nxt = 0   # next own pair to hand to attn@V
            for si in range(NSLOT):
                emit_scores_slot(n, si)
                done += len(SLOT_PAIRS[si])
                if n == 0:
                    if si in QK_SCHED:
                        emit_qk_slice(QK_SCHED[si])
                    for u in V_SCHED.get(si, []):
                        emit_v_pair(u)
                else:
                    emit_head(n, si)
                while nxt <= done - 1 - LAGS[n] and nxt <= 15:
                    emit_proj_batch(n, nxt)
                    nxt += 1
        for p in range(16 - LAGS[3], 16):
            emit_proj_batch(3, p)
        emit_finish_b(3, *emit_finish_a(3))

    nc.compile()
    return nc


def host_prep(x, gamma, beta, Wq, bq, Wk, bk, Wv, bv, Wo, bo):
    """Build the 8 per-core input dicts."""
    f32 = lambda a: np.ascontiguousarray(np.asarray(a, np.float32))
    x = f32(x)
    gamma, beta = f32(gamma), f32(beta)
    Wq, Wk, Wv, Wo = f32(Wq), f32(Wk), f32(Wv), f32(Wo)
    bq, bk, bv, bo = f32(bq), f32(bk), f32(bv), f32(bo)

    wq_dup = np.ascontiguousarray(np.concatenate([Wq, Wq], axis=1))
    z = np.zeros((64, 64), np.float32)
    wk_blk = np.ascontiguousarray(np.block([[Wk, z], [z, Wk]]))
    wv_blk = np.ascontiguousarray(np.block([[Wv, z], [z, Wv]]))
    comb = np.zeros((128, 128), np.float32)
    comb[:64, :64] = 1.0 / 64.0
    comb[64:, 64:] = 1.0 / 64.0
    mdt_np = mybir.dt.np(MDT)
    m = lambda a: np.ascontiguousarray(a).astype(mdt_np)
    shared = {
        "wq": m(wq_dup), "wk": m(wk_blk), "wv": m(wv_blk), "wo": m(Wo),
        "bq": m(np.tile(bq, 2)[None]),
        "bk": m(np.tile(bk, 2)[None]),
        "bv": m(np.tile(bv, 2)[None]),
        "bo": np.ascontiguousarray(bo[:, None]),
        "gam": np.ascontiguousarray(np.tile(gamma, 2)[:, None]),
        "bet": np.ascontiguousarray(np.tile(beta, 2)[:, None]),
        "comb": comb,
    }
    in_maps = []
    for core in range(8):
        b, h = core // 2, core % 2
        xT = x[b].reshape(HW, C).T  # [64, 4096]
        halves = xT.reshape(C, 2, HALF)[:, [h, 1 - h], :]
        xp = np.ascontiguousarray(halves.transpose(1, 0, 2).reshape(128, HALF))
        in_maps.append({"xp": xp, **shared})
    return in_maps


def assemble(results, dtype):
    out = np.empty((B, HW, C), np.float32)
    for core in range(8):
        b, h = core // 2, core % 2
        out[b, HALF * h : HALF * h + HALF] = results[core]["out"].T
    return out.reshape(B, H, W, C).astype(dtype, copy=False)


_NC_CACHE = []


def kernel(x, gamma, beta, Wq, bq, Wk, bk, Wv, bv, Wo, bo):
    from concourse.bass_utils import run_bass_kernel_spmd

    if not _NC_CACHE:
        _NC_CACHE.append(build_nc())
    nc = _NC_CACHE[0]
    in_maps = host_prep(x, gamma, beta, Wq, bq, Wk, bk, Wv, bv, Wo, bo)
    res = run_bass_kernel_spmd(nc, in_maps, core_ids=list(range(8)))
    return assemble(res.results, np.asarray(x).dtype)


if __name__ == "__main__":
    rng = np.random.default_rng(0)
    inputs = {
        "x": rng.standard_normal((B, H, W, C)).astype(np.float32),
        "gamma": np.ones(C, np.float32), "beta": np.zeros(C, np.float32),
        "Wq": (rng.standard_normal((C, C)) / 8).astype(np.float32),
        "bq": np.zeros(C, np.float32),
        "Wk": (rng.standard_normal((C, C)) / 8).astype(np.float32),
        "bk": np.zeros(C, np.float32),
        "Wv": (rng.standard_normal((C, C)) / 8).astype(np.float32),
        "bv": np.zeros(C, np.float32),
        "Wo": (rng.standard_normal((C, C)) / 8).astype(np.float32),
        "bo": np.zeros(C, np.float32),
    }
    out = kernel(**inputs)
    print("kernel ran, out shape", out.shape, out.dtype)



# revision 28
# speedup vs baseline: 1.1643x; 1.1643x over previous
"""Trainium2 Bass kernel for nn_AttentionBlock (B=4, H=W=64, C=64, GroupNorm(8) +
full spatial self-attention), distributed over 8 NeuronCores.

Sharding: core i handles batch b=i//2 and query-half h=i%2 (2048 of the 4096
spatial positions). Each core computes the full GroupNorm and K/V for its
image (cheap) and attention only for its query half. No collectives.

v2 pipeline:
- exp split across THREE engines: ACT (table exp) for some score pairs,
  Pool (gpsimd) and DVE for the rest via a single-op int16 Schraudolph
  (i16 = s*23.083 + 16256.5 truncated, bitcast bf16 ~= e^(s/8), max rel err
  ~4%, final output err ~3e-3; denominator uses the same approximated
  weights so softmax normalization stays consistent).
- PE stream is gap-free: warmup matmuls ramp the clock during GroupNorm
  stats, then per tile scores-pair p / attnV pair p-3 alternate, with
  qk/v production and prev-tile finish matmuls slotted into the bubbles.
- biases: bq/bk folded into the q/k PSUM->SBUF copies (per-partition add);
  bv folded into bo on the host (bv @ Wo + bo).
- softmax denominators ride as a 65th ones-column of V; reciprocal via
  the fast custom-DVE op on [1,512] then broadcast by a bf16 PE matmul.
"""

import sys

sys.path.insert(0, "/opt/trn_rl_repo")

import numpy as np

import concourse.bacc as bacc
import concourse.tile as tile
from concourse import mybir

B, H, W, C = 4, 64, 64, 64
HW = H * W  # 4096
HALF = HW // 2  # 2048
EPS = 1e-5
SCALE = C ** -0.5

F32 = mybir.dt.float32
MDT = mybir.dt.bfloat16  # PE matmul operand dtype
I16 = mybir.dt.int16

# Schraudolph exp in bf16-bit space: i16 = round(s * 2^7/ln2 * SCALE + 127*2^7)
SCH_SCALE = float((2.0 ** 7) / np.log(2.0) * SCALE)
SCH_BIAS = 16251.0  # 127*2^7 shifted -5.5 to center the one-sided
# mantissa-interpolation error (+0..6.7%) around zero

NWARM = 14  # PE warmup matmuls (ramp p-state during GN stats)
LAG = 6     # attnV trails scores by LAG pairs

# engine per exp pair: A=ACT table exp, D=DVE int16-schraudolph. (Pool cannot
# read PSUM on TRN2, so it only gets SBUF->SBUF work: xn, recb, final out.)
EMAP0 = ['D', 'A', 'D', 'A', 'D', 'A', 'A', 'D',
         'A', 'D', 'A', 'D', 'A', 'D', 'A', 'D']      # A8 D8
EMAPN = ['A', 'D', 'A', 'D', 'A', 'D', 'A', 'D',
         'A', 'D', 'D', 'A', 'D', 'D', 'A', 'D']      # A7 D9


def build_nc():
    nc = bacc.Bacc("TRN2", debug=False, num_devices=8)

    # ---- DRAM I/O ----
    xp_d = nc.dram_tensor("xp", [128, HALF], F32, kind="ExternalInput")
    wq_d = nc.dram_tensor("wq", [64, 128], MDT, kind="ExternalInput")
    wk_d = nc.dram_tensor("wk", [128, 128], MDT, kind="ExternalInput")
    wv_d = nc.dram_tensor("wv", [128, 128], MDT, kind="ExternalInput")
    wo_d = nc.dram_tensor("wo", [64, 64], MDT, kind="ExternalInput")
    bq_d = nc.dram_tensor("bq", [128, 1], F32, kind="ExternalInput")
    bo_d = nc.dram_tensor("bo", [128, 1], F32, kind="ExternalInput")
    gam_d = nc.dram_tensor("gam", [128, 1], F32, kind="ExternalInput")
    bet_d = nc.dram_tensor("bet", [128, 1], F32, kind="ExternalInput")
    comb_d = nc.dram_tensor("comb", [128, 128], F32, kind="ExternalInput")
    out_d = nc.dram_tensor("out", [64, HALF], F32, kind="ExternalOutput")

    with tile.TileContext(nc) as tc, \
         tc.tile_pool(name="singles", bufs=1) as singles, \
         tc.tile_pool(name="stats", bufs=1) as stats, \
         tc.tile_pool(name="sc_ps", bufs=2, space="PSUM") as sc_ps, \
         tc.tile_pool(name="pacc_ps", bufs=2, space="PSUM") as pacc_ps, \
         tc.tile_pool(name="aux_ps", bufs=1, space="PSUM") as aux_ps, \
         tc.tile_pool(name="work", bufs=2) as work:

        # ---- input DMAs: x on sync queue, everything else on the ACT hwdge ----
        x_sb = singles.tile([128, HALF], F32)
        for r in range(4):
            nc.sync.dma_start(
                x_sb[:, 512 * r: 512 * r + 512],
                xp_d.ap()[:, 512 * r: 512 * r + 512],
            )
        gam_sb = singles.tile([128, 1], F32)
        nc.scalar.dma_start(gam_sb[:], gam_d.ap())
        bet_sb = singles.tile([128, 1], F32)
        nc.scalar.dma_start(bet_sb[:], bet_d.ap())
        comb_sb = singles.tile([128, 128], F32)
        nc.scalar.dma_start(comb_sb[:], comb_d.ap())
        wq_sb = singles.tile([64, 128], MDT)
        nc.scalar.dma_start(wq_sb[:], wq_d.ap())
        wk_sb = singles.tile([128, 128], MDT)
        nc.scalar.dma_start(wk_sb[:], wk_d.ap())
        bq_sb = singles.tile([128, 1], F32)
        nc.scalar.dma_start(bq_sb[:], bq_d.ap())
        wv_sb = singles.tile([128, 128], MDT)
        nc.scalar.dma_start(wv_sb[:], wv_d.ap())
        wo_sb = singles.tile([64, 64], MDT)
        nc.scalar.dma_start(wo_sb[:], wo_d.ap())
        bo_sb = singles.tile([128, 1], F32)
        nc.scalar.dma_start(bo_sb[:], bo_d.ap())

        # ---- big SBUF tensors ----
        xn_r = singles.tile([128, HALF], MDT)
        q_dup = singles.tile([128, HALF], MDT)
        kt_sb = singles.tile([128, HALF], MDT)
        v_all = singles.tile([128, 65 * 32], MDT)
        attnexp = singles.tile([128, 1024 * 16], MDT)
        out_sb = singles.tile([64, HALF], F32)
        ones_sb = singles.tile([128, 512], MDT)

        # constants on Pool, first thing (warmup matmuls read ones_sb)
        nc.gpsimd.memset(ones_sb[:], 1.0)
        v4 = v_all[:].rearrange("p (h t e) -> p h t e", h=2, e=65)
        nc.gpsimd.memset(v4[:, :, :, 64:65], 1.0)

        # pre-warm the exp ACT table set
        scr = stats.tile([128, 1], F32)
        nc.vector.memset(scr[:], 1.0)
        nc.scalar.activation(scr[:], scr[:], mybir.ActivationFunctionType.Exp)

        # ---- PE warmup: ramp the activity monitor while DVE does GN stats
        # (rides the sc-tag banks, which are free until the first scores) ----
        for w in range(NWARM):
            wps = sc_ps.tile([64, 512], F32, tag="sc", name=f"warm{w}")
            nc.tensor.matmul(wps[:], ones_sb[0:64, 0:64], ones_sb[0:64, :],
                             start=True, stop=True)

        # ---- GroupNorm stats: bn per partition per 512-slice, then a
        # block-diagonal averaging matmul combines across channels ----
        st6 = stats.tile([128, 4, 6], F32)
        mv4 = stats.tile([128, 4, 2], F32)
        for r in range(4):
            nc.vector.bn_stats(st6[:, r, :], x_sb[:, 512 * r: 512 * r + 512])
            nc.vector.bn_aggr(mv4[:, r, :], st6[:, r, :])
        smat = stats.tile([128, 8], F32)  # cols 0-3 mean, 4-7 E[x^2]
        nc.vector.tensor_copy(smat[:, 0:4], mv4[:, :, 0])
        nc.vector.tensor_mul(smat[:, 4:8], mv4[:, :, 0], mv4[:, :, 0])
        nc.vector.tensor_add(smat[:, 4:8], smat[:, 4:8], mv4[:, :, 1])

        cps = pacc_ps.tile([128, 8], F32, tag="pacc")
        nc.tensor.matmul(cps[:], comb_sb[:], smat[:], start=True, stop=True)
        gstat = stats.tile([128, 8], F32)  # 0-3 mean_g, 4-7 E2_g
        nc.vector.tensor_copy(gstat[:], cps[:])

        # var+eps, then rstd = rsqrt via bit-trick seed + Newton steps (DVE)
        ve = stats.tile([128, 4], F32)
        nc.vector.tensor_mul(ve[:], gstat[:, 0:4], gstat[:, 0:4])
        nc.vector.tensor_scalar(
            out=ve[:], in0=ve[:], scalar1=-1.0, scalar2=EPS,
            op0=mybir.AluOpType.mult, op1=mybir.AluOpType.add,
        )
        nc.vector.tensor_add(ve[:], ve[:], gstat[:, 4:8])
        yi = stats.tile([128, 4], mybir.dt.int32)
        nc.vector.tensor_scalar(
            out=yi[:], in0=ve[:].bitcast(mybir.dt.int32), scalar1=1,
            scalar2=None, op0=mybir.AluOpType.logical_shift_right,
        )
        nc.vector.tensor_scalar(
            out=yi[:], in0=yi[:], scalar1=-1, scalar2=0x5F3759DF,
            op0=mybir.AluOpType.mult, op1=mybir.AluOpType.add,
        )
        rstd = stats.tile([128, 4], F32)
        nc.vector.tensor_copy(rstd[:], yi[:].bitcast(F32))
        vh = stats.tile([128, 4], F32)
        nc.vector.tensor_scalar_mul(vh[:], ve[:], -0.5)
        t_nw = stats.tile([128, 4], F32)
        for _ in range(2):
            nc.vector.tensor_mul(t_nw[:], rstd[:], rstd[:])
            nc.vector.tensor_mul(t_nw[:], t_nw[:], vh[:])
            nc.vector.tensor_scalar(
                out=t_nw[:], in0=t_nw[:], scalar1=1.0, scalar2=1.5,
                op0=mybir.AluOpType.mult, op1=mybir.AluOpType.add,
            )
            nc.vector.tensor_mul(rstd[:], rstd[:], t_nw[:])

        gsc = stats.tile([128, 4], F32)
        nc.vector.tensor_scalar_mul(gsc[:], rstd[:], gam_sb[:])
        gbias = stats.tile([128, 4], F32)
        nc.vector.tensor_mul(gbias[:], gstat[:, 0:4], gsc[:])
        nc.vector.tensor_scalar(
            out=gbias[:], in0=gbias[:], scalar1=-1.0, scalar2=bet_sb[:],
            op0=mybir.AluOpType.mult, op1=mybir.AluOpType.add,
        )
        # the fp32 residual pass folds in bo (bo rides rows 0:63 of the bias;
        # rows 64:127 of x_sb are never read again after the qkv matmuls)
        gbias2 = stats.tile([128, 4], F32)
        nc.vector.tensor_scalar_add(gbias2[:], gbias[:], bo_sb[:])
        # xn = x * gsc + gbias: both the bf16 matmul copy and the fp32
        # residual pass run on Pool (SBUF->SBUF), keeping ACT/DVE free for
        # the PSUM-side attention work. Per slice: bf16 first, then the
        # in-place fp32 overwrite (same engine => ordered).
        for r in range(4):
            sl = slice(512 * r, 512 * r + 512)
            nc.gpsimd.tensor_scalar(
                out=xn_r[:, sl], in0=x_sb[:, sl],
                scalar1=gsc[:, r: r + 1], scalar2=gbias[:, r: r + 1],
                op0=mybir.AluOpType.mult, op1=mybir.AluOpType.add,
            )
            nc.gpsimd.tensor_scalar(
                out=x_sb[:, sl], in0=x_sb[:, sl],
                scalar1=gsc[:, r: r + 1], scalar2=gbias2[:, r: r + 1],
                op0=mybir.AluOpType.mult, op1=mybir.AluOpType.add,
            )

        # ---- emission helpers ----
        def emit_qk_slice(t, pool_tags):
            # k^T packed by half (lhsT = blockdiag(Wk, Wk)); q^T duplicated on
            # both partition halves (lhsT = [Wq | Wq]). bk is dropped exactly
            # (a per-query score constant cancels in softmax); bq folds into
            # the q copy as a per-partition bias on ACT.
            sl = slice(512 * t, 512 * t + 512)
            pool_k, tag_k = pool_tags[0]
            pool_q, tag_q = pool_tags[1]
            ps2 = pool_k.tile([128, 512], F32, tag=tag_k, name=f"kps{t}")
            nc.tensor.matmul(ps2[:], wk_sb[:], xn_r[:, sl], start=True,
                             stop=True)
            nc.vector.tensor_copy(kt_sb[:, sl], ps2[:])
            ps = pool_q.tile([128, 512], F32, tag=tag_q, name=f"qps{t}")
            nc.tensor.matmul(ps[:], wq_sb[:], xn_r[0:64, sl], start=True,
                             stop=True)
            nc.scalar.activation(
                q_dup[:, sl], ps[:], mybir.ActivationFunctionType.Identity,
                bias=bq_sb[:],
            )

        def emit_v(u, pool_tag=None):
            # v position-major, two 128-position chunks (halves) per matmul;
            # single copy with a dual-chunk strided output AP. Even u on the
            # bcq psum tag + DVE copy, odd u on fpq + ACT.
            sl = slice(128 * u, 128 * u + 128)
            pool, tag = pool_tag or (aux_ps, "bcq" if u % 2 == 0 else "fpq")
            ps = pool.tile([128, 128], F32, tag=tag, name=f"vps{u}")
            nc.tensor.matmul(ps[:], xn_r[:, sl], wv_sb[:], start=True,
                             stop=True)
            psr = ps[:].rearrange("p (h e) -> p h e", h=2)
            if u % 2 == 0:
                nc.vector.tensor_copy(v4[:, :, u, 0:64], psr[:, :, :])
            else:
                nc.scalar.activation(
                    v4[:, :, u, 0:64], psr[:, :, :],
                    mybir.ActivationFunctionType.Identity,
                )

        def emit_scores(n, p):
            # pair p: kv chunks p (half0, PE rows 0-63) and p+16 (half1, rows
            # 64-127) run concurrently; one [128,1024] 2-bank psum tile
            qsl = slice(512 * n, 512 * n + 512)
            ksl = slice(128 * p, 128 * p + 128)
            ps = sc_ps.tile([128, 1024], F32, tag="sc", name=f"sc{n}_{p}")
            nc.tensor.matmul(ps[:, 0:512], kt_sb[0:64, ksl],
                             q_dup[0:64, qsl], start=True, stop=True)
            nc.tensor.matmul(ps[:, 512:1024], kt_sb[64:128, ksl],
                             q_dup[64:128, qsl], start=True, stop=True)
            return ps

        def emit_exp(n, p, ps):
            # attnexp layout pair-major: chunk p at 1024p, chunk p+16 at
            # 1024p+512 — both written by this single instruction
            dst = attnexp[:, 1024 * p: 1024 * p + 1024]
            e = (EMAP0 if n == 0 else EMAPN)[p]
            if e == 'A':
                nc.scalar.activation(dst, ps[:],
                                     mybir.ActivationFunctionType.Exp,
                                     scale=SCALE)
            else:
                nc.vector.tensor_scalar(
                    out=dst.bitcast(I16), in0=ps[:],
                    scalar1=SCH_SCALE, scalar2=SCH_BIAS,
                    op0=mybir.AluOpType.mult, op1=mybir.AluOpType.add,
                )

        paccs = {}

        def emit_attnv(n, p):
            # kv chunk pair (p, p+16) — consumes exp pair p
            if n not in paccs:
                paccs[n] = pacc_ps.tile([65, 512], F32, tag="pacc",
                                        name=f"pacc{n}")
            pacc = paccs[n]
            for t in (p, p + 16):
                off = 1024 * p + (512 if t >= 16 else 0)
                nc.tensor.matmul(
                    pacc[:], v_all[:, 65 * t: 65 * t + 65],
                    attnexp[:, off: off + 512],
                    start=(t == 0), stop=(t == 31),
                )

        # finish chain for tile n, split into steps scheduled across pairs of
        # tile n+1 so the PE stream stays dense
        fin = {}

        def fin_a(n):
            # free the PSUM accumulator ASAP: unnormalized proj rows (bf16)
            # on ACT; the raw denominator row stays in PSUM for fin_b's rec
            pacc = paccs[n]
            projn_u = work.tile([64, 512], MDT, tag="projn", name=f"pn{n}")
            nc.scalar.activation(projn_u[:], pacc[0:64, :],
                                 mybir.ActivationFunctionType.Identity)
            fin[n] = (projn_u,)

        def fin_b(n):
            # per-query 1/denom straight off PSUM (fast custom-DVE approx).
            # The custom op ignores input partition offsets, so run it over
            # all 65 partitions (same cost: DVE time = free size) and use
            # row 64. Then a tiny bf16 convert on Pool for the broadcast.
            pacc = paccs.pop(n)
            (projn_u,) = fin[n]
            rec = work.tile([65, 512], F32, tag="rec", name=f"rec{n}")
            nc.vector.reciprocal_approx_fast(out=rec[:], in_=pacc[:, :])
            recb = work.tile([1, 512], MDT, tag="recb", name=f"recb{n}")
            nc.gpsimd.tensor_copy(recb[:], rec[64:65, :])
            fin[n] = (projn_u, recb)

        def fin_c(n):
            # PE: broadcast 1/denom to [64,512] + out-projection
            projn_u, recb = fin[n]
            bc_ps = aux_ps.tile([64, 512], F32, tag="bcq", name=f"bc{n}")
            nc.tensor.matmul(bc_ps[:], ones_sb[0:1, 0:64], recb[:],
                             start=True, stop=True)
            fps = aux_ps.tile([64, 512], F32, tag="fpq", name=f"fps{n}")
            nc.tensor.matmul(fps[:], wo_sb[:], projn_u[:], start=True,
                             stop=True)
            bc_sb = work.tile([64, 512], F32, tag="bc", name=f"bcs{n}")
            nc.scalar.activation(bc_sb[:], bc_ps[:],
                                 mybir.ActivationFunctionType.Identity)
            fin[n] = (fps, bc_sb)

        def fin_d(n):
            # normalize on DVE, then bias + residual + store on Pool
            fps, bc_sb = fin.pop(n)
            qsl = slice(512 * n, 512 * n + 512)
            mn = work.tile([64, 512], F32, tag="mn", name=f"mn{n}")
            nc.vector.tensor_mul(mn[:], fps[:], bc_sb[:])
            nc.gpsimd.tensor_add(out_sb[:, qsl], mn[:], x_sb[0:64, qsl])
            nc.sync.dma_start(out_d.ap()[:, qsl], out_sb[:, qsl])

        # ---- software-pipelined attention ----
        # tile 0 extras: qk slices 1-3 and v chunks produced just in time
        # (scores pair p needs kt slice p//4, attnV pair p-LAG needs v chunk
        # p-LAG). The earliest qkv psums ride the pacc-tag banks (free until
        # the first pacc allocation at p=LAG); the rest alternate bcq/fpq so
        # every tenant's copy has >= 2 pairs to drain before bank reuse.
        PACC_TAG = (pacc_ps, "pacc")
        T0_EXTRA = {0: [("qk", 1, (PACC_TAG, PACC_TAG))],
                    1: [("v", 0, PACC_TAG), ("v", 1, PACC_TAG)],
                    2: [("v", 2, None), ("v", 3, None)],
                    3: [("qk", 2, None)],
                    4: [("v", 4, None), ("v", 5, None)],
                    5: [("v", 6, None)], 6: [("v", 7, None)],
                    7: [("qk", 3, None)],
                    8: [("v", 8, None)], 9: [("v", 9, None)],
                    10: [("v", 10, None)], 11: [("v", 11, None)],
                    12: [("v", 12, None)], 13: [("v", 13, None)],
                    14: [("v", 14, None)], 15: [("v", 15, None)]}
        # tiles 1-3: previous tile's spill attnV pairs + finish steps (spread
        # out so each step's engine work has slack before its consumer)
        TN_EXTRA = {0: [("spill", 10)], 1: [("spill", 11)],
                    2: [("spill", 12)], 3: [("spill", 13)],
                    4: [("spill", 14)], 5: [("spill", 15)],
                    6: [("fina",)], 7: [("finb",)],
                    9: [("finc",)], 11: [("find",)]}
        AUX = ((aux_ps, "bcq"), (aux_ps, "fpq"))

        emit_qk_slice(0, AUX)
        for n in range(4):
            for p in range(16):
                ps = emit_scores(n, p)
                if p >= LAG:
                    emit_attnv(n, p - LAG)
                if n == 0:
                    for item in T0_EXTRA.get(p, []):
                        if item[0] == "qk":
                            emit_qk_slice(item[1], item[2] or AUX)
                        else:
                            emit_v(item[1], item[2])
                else:
                    for item in TN_EXTRA.get(p, []):
                        if item[0] == "spill":
                            emit_attnv(n - 1, item[1])
                        elif item[0] == "fina":
                            fin_a(n - 1)
                        elif item[0] == "finb":
                            fin_b(n - 1)
                        elif item[0] == "finc":
                            fin_c(n - 1)
                        else:
                            fin_d(n - 1)
                emit_exp(n, p, ps)
        for p in range(16 - LAG, 16):
            emit_attnv(3, p)
        fin_a(3)
        fin_b(3)
        fin_c(3)
        fin_d(3)

    nc.compile()
    return nc


def host_prep(x, gamma, beta, Wq, bq, Wk, bk, Wv, bv, Wo, bo):
    """Build the 8 per-core input dicts."""
    f32 = lambda a: np.ascontiguousarray(np.asarray(a, np.float32))
    x = f32(x)
    gamma, beta = f32(gamma), f32(beta)
    Wq, Wk, Wv, Wo = f32(Wq), f32(Wk), f32(Wv), f32(Wo)
    bq, bk, bv, bo = f32(bq), f32(bk), f32(bv), f32(bo)

    wq_dup = np.ascontiguousarray(np.concatenate([Wq, Wq], axis=1))
    z = np.zeros((64, 64), np.float32)
    wk_blk = np.ascontiguousarray(np.block([[Wk, z], [z, Wk]]))
    wv_blk = np.ascontiguousarray(np.block([[Wv, z], [z, Wv]]))
    comb = np.zeros((128, 128), np.float32)
    comb[:64, :64] = 1.0 / 64.0
    comb[64:, 64:] = 1.0 / 64.0
    bo_f = bv @ Wo + bo  # fold v bias through the out-projection
    mdt_np = mybir.dt.np(MDT)
    m = lambda a: np.ascontiguousarray(a).astype(mdt_np)
    shared = {
        "wq": m(wq_dup), "wk": m(wk_blk), "wv": m(wv_blk), "wo": m(Wo),
        "bq": np.ascontiguousarray(np.tile(bq, 2)[:, None]),
        "bo": np.ascontiguousarray(
            np.concatenate([bo_f, np.zeros(64, np.float32)])[:, None]),
        "gam": np.ascontiguousarray(np.tile(gamma, 2)[:, None]),
        "bet": np.ascontiguousarray(np.tile(beta, 2)[:, None]),
        "comb": comb,
    }
    in_maps = []
    for core in range(8):
        b, h = core // 2, core % 2
        xT = x[b].reshape(HW, C).T  # [64, 4096]
        halves = xT.reshape(C, 2, HALF)[:, [h, 1 - h], :]
        xp = np.ascontiguousarray(halves.transpose(1, 0, 2).reshape(128, HALF))
        in_maps.append({"xp": xp, **shared})
    return in_maps


def assemble(results, dtype):
    out = np.empty((B, HW, C), np.float32)
    for core in range(8):
        b, h = core // 2, core % 2
        out[b, HALF * h: HALF * h + HALF] = results[core]["out"].T
    return out.reshape(B, H, W, C).astype(dtype, copy=False)


_NC_CACHE = []


def kernel(x, gamma, beta, Wq, bq, Wk, bk, Wv, bv, Wo, bo):
    from concourse.bass_utils import run_bass_kernel_spmd

    if not _NC_CACHE:
        _NC_CACHE.append(build_nc())
    nc = _NC_CACHE[0]
    in_maps = host_prep(x, gamma, beta, Wq, bq, Wk, bk, Wv, bv, Wo, bo)
    res = run_bass_kernel_spmd(nc, in_maps, core_ids=list(range(8)))
    return assemble(res.results, np.asarray(x).dtype)


if __name__ == "__main__":
    rng = np.random.default_rng(0)
    inputs = {
        "x": rng.standard_normal((B, H, W, C)).astype(np.float32),
        "gamma": np.ones(C, np.float32), "beta": np.zeros(C, np.float32),
        "Wq": (rng.standard_normal((C, C)) / 8).astype(np.float32),
        "bq": np.zeros(C, np.float32),
        "Wk": (rng.standard_normal((C, C)) / 8).astype(np.float32),
        "bk": np.zeros(C, np.float32),
        "Wv": (rng.standard_normal((C, C)) / 8).astype(np.float32),
        "bv": np.zeros(C, np.float32),
        "Wo": (rng.standard_normal((C, C)) / 8).astype(np.float32),
        "bo": np.zeros(C, np.float32),
    }
    out = kernel(**inputs)
    print("kernel ran, out shape", out.shape, out.dtype)


# revision 31
# speedup vs baseline: 1.3664x; 1.1736x over previous
"""Trainium2 Bass kernel for nn_AttentionBlock (B=4, H=W=64, C=64, GroupNorm(8) +
full spatial self-attention), distributed over 8 NeuronCores.

Sharding: core i handles batch b=i//2 and query-half h=i%2 (2048 of the 4096
spatial positions). Each core computes the full GroupNorm and K/V for its
image (cheap) and attention only for its query half. No collectives.

v2 pipeline:
- exp split across THREE engines: ACT (table exp) for some score pairs,
  Pool (gpsimd) and DVE for the rest via a single-op int16 Schraudolph
  (i16 = s*23.083 + 16256.5 truncated, bitcast bf16 ~= e^(s/8), max rel err
  ~4%, final output err ~3e-3; denominator uses the same approximated
  weights so softmax normalization stays consistent).
- PE stream is gap-free: warmup matmuls ramp the clock during GroupNorm
  stats, then per tile scores-pair p / attnV pair p-3 alternate, with
  qk/v production and prev-tile finish matmuls slotted into the bubbles.
- biases: bq/bk folded into the q/k PSUM->SBUF copies (per-partition add);
  bv folded into bo on the host (bv @ Wo + bo).
- softmax denominators ride as a 65th ones-column of V; reciprocal via
  the fast custom-DVE op on [1,512] then broadcast by a bf16 PE matmul.
"""

import sys

sys.path.insert(0, "/opt/trn_rl_repo")

import numpy as np

import concourse.bacc as bacc
import concourse.tile as tile
from concourse import mybir

B, H, W, C = 4, 64, 64, 64
HW = H * W  # 4096
HALF = HW // 2  # 2048
EPS = 1e-5
SCALE = C ** -0.5

F32 = mybir.dt.float32
MDT = mybir.dt.bfloat16  # PE matmul operand dtype
I16 = mybir.dt.int16

# Schraudolph exp in bf16-bit space: i16 = round(s * 2^7/ln2 * SCALE + 127*2^7)
SCH_SCALE = float((2.0 ** 7) / np.log(2.0) * SCALE)
SCH_BIAS = 16251.0  # 127*2^7 shifted -5.5 to center the one-sided
# mantissa-interpolation error (+0..6.7%) around zero

NWARM = 24  # PE warmup matmuls (ramp p-state during GN stats)
LAG = 6     # attnV trails scores by LAG pairs

# engine per exp pair: A=ACT table exp, D=DVE int16-schraudolph. (Pool cannot
# read PSUM on TRN2, so it only gets SBUF->SBUF work: xn, recb, final out.)
# Tile tails lean A so DVE is clear for the next tile's start.
EMAP0 = ['D', 'A', 'D', 'A', 'D', 'A', 'D', 'A',
         'D', 'A', 'D', 'A', 'A', 'D', 'A', 'A']      # A9 D7
EMAPN = ['D', 'A', 'D', 'A', 'D', 'A', 'D', 'A',
         'D', 'D', 'A', 'D', 'D', 'A', 'A', 'A']      # A8 D8


def build_nc():
    nc = bacc.Bacc("TRN2", debug=False, num_devices=8)

    # ---- DRAM I/O ----
    xp_d = nc.dram_tensor("xp", [128, HALF], F32, kind="ExternalInput")
    wq_d = nc.dram_tensor("wq", [64, 128], MDT, kind="ExternalInput")
    wk_d = nc.dram_tensor("wk", [128, 128], MDT, kind="ExternalInput")
    wv_d = nc.dram_tensor("wv", [128, 128], MDT, kind="ExternalInput")
    wo_d = nc.dram_tensor("wo", [64, 64], MDT, kind="ExternalInput")
    bq_d = nc.dram_tensor("bq", [128, 1], F32, kind="ExternalInput")
    bo_d = nc.dram_tensor("bo", [128, 1], F32, kind="ExternalInput")
    gam_d = nc.dram_tensor("gam", [128, 1], F32, kind="ExternalInput")
    bet_d = nc.dram_tensor("bet", [128, 1], F32, kind="ExternalInput")
    comb_d = nc.dram_tensor("comb", [128, 128], F32, kind="ExternalInput")
    out_d = nc.dram_tensor("out", [64, HALF], F32, kind="ExternalOutput")

    with tile.TileContext(nc) as tc, \
         tc.tile_pool(name="singles", bufs=1) as singles, \
         tc.tile_pool(name="stats", bufs=1) as stats, \
         tc.tile_pool(name="sc_ps", bufs=2, space="PSUM") as sc_ps, \
         tc.tile_pool(name="pacc_ps", bufs=2, space="PSUM") as pacc_ps, \
         tc.tile_pool(name="aux_ps", bufs=1, space="PSUM") as aux_ps, \
         tc.tile_pool(name="work", bufs=2) as work:

        # ---- input DMAs: x alternates across the two hardware queues so the
        # GN stats chain starts sooner; weights follow on the ACT queue ----
        x_sb = singles.tile([128, HALF], F32)
        for r in range(4):
            eng = nc.sync if r % 2 == 0 else nc.scalar
            eng.dma_start(
                x_sb[:, 512 * r: 512 * r + 512],
                xp_d.ap()[:, 512 * r: 512 * r + 512],
            )
        gam_sb = singles.tile([128, 1], F32)
        nc.scalar.dma_start(gam_sb[:], gam_d.ap())
        bet_sb = singles.tile([128, 1], F32)
        nc.scalar.dma_start(bet_sb[:], bet_d.ap())
        comb_sb = singles.tile([128, 128], F32)
        nc.scalar.dma_start(comb_sb[:], comb_d.ap())
        wq_sb = singles.tile([64, 128], MDT)
        nc.scalar.dma_start(wq_sb[:], wq_d.ap())
        wk_sb = singles.tile([128, 128], MDT)
        nc.scalar.dma_start(wk_sb[:], wk_d.ap())
        bq_sb = singles.tile([128, 1], F32)
        nc.scalar.dma_start(bq_sb[:], bq_d.ap())
        wv_sb = singles.tile([128, 128], MDT)
        nc.scalar.dma_start(wv_sb[:], wv_d.ap())
        wo_sb = singles.tile([64, 64], MDT)
        nc.scalar.dma_start(wo_sb[:], wo_d.ap())
        bo_sb = singles.tile([128, 1], F32)
        nc.scalar.dma_start(bo_sb[:], bo_d.ap())

        # ---- big SBUF tensors ----
        xn_r = singles.tile([128, HALF], MDT)
        q_dup = singles.tile([128, HALF], MDT)
        kt_sb = singles.tile([128, HALF], MDT)
        v_all = singles.tile([128, 65 * 32], MDT)
        attnexp = singles.tile([128, 1024 * 16], MDT)
        out_sb = singles.tile([64, HALF], F32)
        ones_sb = singles.tile([128, 512], MDT)

        # constants on Pool, first thing (warmup matmuls read ones_sb)
        nc.gpsimd.memset(ones_sb[:], 1.0)
        v4 = v_all[:].rearrange("p (h t e) -> p h t e", h=2, e=65)
        nc.gpsimd.memset(v4[:, :, :, 64:65], 1.0)

        # pre-warm the exp ACT table set
        scr = stats.tile([128, 1], F32)
        nc.vector.memset(scr[:], 1.0)
        nc.scalar.activation(scr[:], scr[:], mybir.ActivationFunctionType.Exp)

        # ---- PE warmup: ramp the activity monitor while DVE does GN stats
        # (rides the sc-tag banks, which are free until the first scores) ----
        for w in range(NWARM):
            wps = sc_ps.tile([128, 512], F32, tag="sc", name=f"warm{w}")
            nc.tensor.matmul(wps[:], ones_sb[:, 0:128], ones_sb[:, :],
                             start=True, stop=True)

        # ---- GroupNorm stats: bn per partition per 512-slice, then a
        # block-diagonal averaging matmul combines across channels ----
        st6 = stats.tile([128, 4, 6], F32)
        mv4 = stats.tile([128, 4, 2], F32)
        for r in range(4):
            nc.vector.bn_stats(st6[:, r, :], x_sb[:, 512 * r: 512 * r + 512])
            nc.vector.bn_aggr(mv4[:, r, :], st6[:, r, :])
        smat = stats.tile([128, 8], F32)  # cols 0-3 mean, 4-7 E[x^2]
        nc.vector.tensor_copy(smat[:, 0:4], mv4[:, :, 0])
        nc.vector.tensor_mul(smat[:, 4:8], mv4[:, :, 0], mv4[:, :, 0])
        nc.vector.tensor_add(smat[:, 4:8], smat[:, 4:8], mv4[:, :, 1])

        cps = pacc_ps.tile([128, 8], F32, tag="pacc")
        nc.tensor.matmul(cps[:], comb_sb[:], smat[:], start=True, stop=True)
        gstat = stats.tile([128, 8], F32)  # 0-3 mean_g, 4-7 E2_g
        nc.vector.tensor_copy(gstat[:], cps[:])

        # var+eps, then rstd = rsqrt via bit-trick seed + Newton steps (DVE)
        ve = stats.tile([128, 4], F32)
        nc.vector.tensor_mul(ve[:], gstat[:, 0:4], gstat[:, 0:4])
        nc.vector.tensor_scalar(
            out=ve[:], in0=ve[:], scalar1=-1.0, scalar2=EPS,
            op0=mybir.AluOpType.mult, op1=mybir.AluOpType.add,
        )
        nc.vector.tensor_add(ve[:], ve[:], gstat[:, 4:8])
        yi = stats.tile([128, 4], mybir.dt.int32)
        nc.vector.tensor_scalar(
            out=yi[:], in0=ve[:].bitcast(mybir.dt.int32), scalar1=1,
            scalar2=None, op0=mybir.AluOpType.logical_shift_right,
        )
        nc.vector.tensor_scalar(
            out=yi[:], in0=yi[:], scalar1=-1, scalar2=0x5F3759DF,
            op0=mybir.AluOpType.mult, op1=mybir.AluOpType.add,
        )
        rstd = stats.tile([128, 4], F32)
        nc.vector.tensor_copy(rstd[:], yi[:].bitcast(F32))
        vh = stats.tile([128, 4], F32)
        nc.vector.tensor_scalar_mul(vh[:], ve[:], -0.5)
        t_nw = stats.tile([128, 4], F32)
        for _ in range(2):
            nc.vector.tensor_mul(t_nw[:], rstd[:], rstd[:])
            nc.vector.tensor_mul(t_nw[:], t_nw[:], vh[:])
            nc.vector.tensor_scalar(
                out=t_nw[:], in0=t_nw[:], scalar1=1.0, scalar2=1.5,
                op0=mybir.AluOpType.mult, op1=mybir.AluOpType.add,
            )
            nc.vector.tensor_mul(rstd[:], rstd[:], t_nw[:])

        gsc = stats.tile([128, 4], F32)
        nc.vector.tensor_scalar_mul(gsc[:], rstd[:], gam_sb[:])
        gbias = stats.tile([128, 4], F32)
        nc.vector.tensor_mul(gbias[:], gstat[:, 0:4], gsc[:])
        nc.vector.tensor_scalar(
            out=gbias[:], in0=gbias[:], scalar1=-1.0, scalar2=bet_sb[:],
            op0=mybir.AluOpType.mult, op1=mybir.AluOpType.add,
        )
        # the fp32 residual pass folds in bo (bo rides rows 0:63 of the bias;
        # rows 64:127 of x_sb are never read again after the qkv matmuls)
        gbias2 = stats.tile([128, 4], F32)
        nc.vector.tensor_scalar_add(gbias2[:], gbias[:], bo_sb[:])
        # xn = x * gsc + gbias: both the bf16 matmul copy and the fp32
        # residual pass run on Pool (SBUF->SBUF), keeping ACT/DVE free for
        # the PSUM-side attention work. Per slice: bf16 first, then the
        # in-place fp32 overwrite (same engine => ordered).
        for r in range(4):
            sl = slice(512 * r, 512 * r + 512)
            nc.gpsimd.tensor_scalar(
                out=xn_r[:, sl], in0=x_sb[:, sl],
                scalar1=gsc[:, r: r + 1], scalar2=gbias[:, r: r + 1],
                op0=mybir.AluOpType.mult, op1=mybir.AluOpType.add,
            )
            nc.gpsimd.tensor_scalar(
                out=x_sb[:, sl], in0=x_sb[:, sl],
                scalar1=gsc[:, r: r + 1], scalar2=gbias2[:, r: r + 1],
                op0=mybir.AluOpType.mult, op1=mybir.AluOpType.add,
            )

        # ---- emission helpers ----
        def emit_qk_slice(t, pool_tags):
            # k^T packed by half (lhsT = blockdiag(Wk, Wk)); q^T duplicated on
            # both partition halves (lhsT = [Wq | Wq]). bk is dropped exactly
            # (a per-query score constant cancels in softmax); bq folds into
            # the q copy as a per-partition bias on ACT.
            sl = slice(512 * t, 512 * t + 512)
            pool_k, tag_k = pool_tags[0]
            pool_q, tag_q = pool_tags[1]
            ps2 = pool_k.tile([128, 512], F32, tag=tag_k, name=f"kps{t}")
            nc.tensor.matmul(ps2[:], wk_sb[:], xn_r[:, sl], start=True,
                             stop=True)
            nc.vector.tensor_copy(kt_sb[:, sl], ps2[:])
            ps = pool_q.tile([128, 512], F32, tag=tag_q, name=f"qps{t}")
            nc.tensor.matmul(ps[:], wq_sb[:], xn_r[0:64, sl], start=True,
                             stop=True)
            nc.scalar.activation(
                q_dup[:, sl], ps[:], mybir.ActivationFunctionType.Identity,
                bias=bq_sb[:],
            )

        def emit_v(u, pool_tag=None):
            # v position-major, two 128-position chunks (halves) per matmul;
            # single copy with a dual-chunk strided output AP. Even u on the
            # bcq psum tag + DVE copy, odd u on fpq + ACT.
            sl = slice(128 * u, 128 * u + 128)
            pool, tag = pool_tag or (aux_ps, "bcq" if u % 2 == 0 else "fpq")
            ps = pool.tile([128, 128], F32, tag=tag, name=f"vps{u}")
            nc.tensor.matmul(ps[:], xn_r[:, sl], wv_sb[:], start=True,
                             stop=True)
            psr = ps[:].rearrange("p (h e) -> p h e", h=2)
            if u % 2 == 0:
                nc.vector.tensor_copy(v4[:, :, u, 0:64], psr[:, :, :])
            else:
                nc.scalar.activation(
                    v4[:, :, u, 0:64], psr[:, :, :],
                    mybir.ActivationFunctionType.Identity,
                )

        def emit_scores(n, p):
            # pair p: kv chunks p (half0, PE rows 0-63) and p+16 (half1, rows
            # 64-127) run concurrently; one [128,1024] 2-bank psum tile
            qsl = slice(512 * n, 512 * n + 512)
            ksl = slice(128 * p, 128 * p + 128)
            ps = sc_ps.tile([128, 1024], F32, tag="sc", name=f"sc{n}_{p}")
            nc.tensor.matmul(ps[:, 0:512], kt_sb[0:64, ksl],
                             q_dup[0:64, qsl], start=True, stop=True)
            nc.tensor.matmul(ps[:, 512:1024], kt_sb[64:128, ksl],
                             q_dup[64:128, qsl], start=True, stop=True)
            return ps

        def emit_exp(n, p, ps):
            # attnexp layout pair-major: chunk p at 1024p, chunk p+16 at
            # 1024p+512 — both written by this single instruction
            dst = attnexp[:, 1024 * p: 1024 * p + 1024]
            e = (EMAP0 if n == 0 else EMAPN)[p]
            if e == 'A':
                nc.scalar.activation(dst, ps[:],
                                     mybir.ActivationFunctionType.Exp,
                                     scale=SCALE)
            else:
                nc.vector.tensor_scalar(
                    out=dst.bitcast(I16), in0=ps[:],
                    scalar1=SCH_SCALE, scalar2=SCH_BIAS,
                    op0=mybir.AluOpType.mult, op1=mybir.AluOpType.add,
                )

        paccs = {}

        def emit_attnv(n, p):
            # kv chunk pair (p, p+16) — consumes exp pair p
            if n not in paccs:
                paccs[n] = pacc_ps.tile([65, 512], F32, tag="pacc",
                                        name=f"pacc{n}")
            pacc = paccs[n]
            for t in (p, p + 16):
                off = 1024 * p + (512 if t >= 16 else 0)
                nc.tensor.matmul(
                    pacc[:], v_all[:, 65 * t: 65 * t + 65],
                    attnexp[:, off: off + 512],
                    start=(t == 0), stop=(t == 31),
                )

        # finish chain for tile n, split into steps scheduled across pairs of
        # tile n+1 so the PE stream stays dense
        fin = {}

        def fin_a(n):
            # free the PSUM accumulator ASAP: unnormalized proj rows (bf16)
            # on ACT; the raw denominator row stays in PSUM for fin_b's rec
            pacc = paccs[n]
            projn_u = work.tile([64, 512], MDT, tag="projn", name=f"pn{n}")
            nc.scalar.activation(projn_u[:], pacc[0:64, :],
                                 mybir.ActivationFunctionType.Identity)
            fin[n] = (projn_u,)

        def fin_b(n):
            # per-query 1/denom straight off PSUM (fast custom-DVE approx).
            # The custom op ignores input partition offsets, so run it over
            # all 65 partitions (same cost: DVE time = free size) and use
            # row 64. Then a tiny bf16 convert on Pool for the broadcast.
            pacc = paccs.pop(n)
            (projn_u,) = fin[n]
            rec = work.tile([65, 512], F32, tag="rec", name=f"rec{n}")
            nc.vector.reciprocal_approx_fast(out=rec[:], in_=pacc[:, :])
            recb = work.tile([1, 512], MDT, tag="recb", name=f"recb{n}")
            nc.gpsimd.tensor_copy(recb[:], rec[64:65, :])
            fin[n] = (projn_u, recb)

        def fin_c(n):
            # PE: broadcast 1/denom to [64,512] + out-projection
            projn_u, recb = fin[n]
            bc_ps = aux_ps.tile([64, 512], F32, tag="bcq", name=f"bc{n}")
            nc.tensor.matmul(bc_ps[:], ones_sb[0:1, 0:64], recb[:],
                             start=True, stop=True)
            fps = aux_ps.tile([64, 512], F32, tag="fpq", name=f"fps{n}")
            nc.tensor.matmul(fps[:], wo_sb[:], projn_u[:], start=True,
                             stop=True)
            bc_sb = work.tile([64, 512], F32, tag="bc", name=f"bcs{n}")
            nc.scalar.activation(bc_sb[:], bc_ps[:],
                                 mybir.ActivationFunctionType.Identity)
            fin[n] = (fps, bc_sb)

        def fin_d(n):
            # normalize on DVE, then bias + residual + store on Pool
            fps, bc_sb = fin.pop(n)
            qsl = slice(512 * n, 512 * n + 512)
            mn = work.tile([64, 512], F32, tag="mn", name=f"mn{n}")
            nc.vector.tensor_mul(mn[:], fps[:], bc_sb[:])
            nc.gpsimd.tensor_add(out_sb[:, qsl], mn[:], x_sb[0:64, qsl])
            nc.sync.dma_start(out_d.ap()[:, qsl], out_sb[:, qsl])

        # ---- software-pipelined attention ----
        # tile 0 extras: qk slices 1-3 and v chunks produced just in time
        # (scores pair p needs kt slice p//4, attnV pair p-LAG needs v chunk
        # p-LAG). The earliest qkv psums ride the pacc-tag banks (free until
        # the first pacc allocation at p=LAG); the rest alternate bcq/fpq so
        # every tenant's copy has >= 2 pairs to drain before bank reuse.
        PACC_TAG = (pacc_ps, "pacc")
        T0_EXTRA = {0: [("qk", 1, (PACC_TAG, PACC_TAG))],
                    1: [("v", 0, PACC_TAG), ("v", 1, PACC_TAG)],
                    2: [("v", 2, None), ("v", 3, None)],
                    3: [("qk", 2, None)],
                    4: [("v", 4, None), ("v", 5, None)],
                    5: [("v", 6, None)], 6: [("v", 7, None)],
                    7: [("qk", 3, None)],
                    8: [("v", 8, None)], 9: [("v", 9, None)],
                    10: [("v", 10, None)], 11: [("v", 11, None)],
                    12: [("v", 12, None)], 13: [("v", 13, None)],
                    14: [("v", 14, None)], 15: [("v", 15, None)]}
        # tiles 1-3: previous tile's spill attnV pairs + finish steps (spread
        # out so each step's engine work has slack before its consumer)
        TN_EXTRA = {0: [("spill", 10)], 1: [("spill", 11)],
                    2: [("spill", 12)], 3: [("spill", 13)],
                    4: [("spill", 14)], 5: [("spill", 15)],
                    6: [("fina",)], 7: [("finb",)],
                    9: [("finc",)], 11: [("find",)]}
        AUX = ((aux_ps, "bcq"), (aux_ps, "fpq"))

        emit_qk_slice(0, AUX)
        for n in range(4):
            for p in range(16):
                ps = emit_scores(n, p)
                if p >= LAG:
                    emit_attnv(n, p - LAG)
                if n == 0:
                    for item in T0_EXTRA.get(p, []):
                        if item[0] == "qk":
                            emit_qk_slice(item[1], item[2] or AUX)
                        else:
                            emit_v(item[1], item[2])
                else:
                    for item in TN_EXTRA.get(p, []):
                        if item[0] == "spill":
                            emit_attnv(n - 1, item[1])
                        elif item[0] == "fina":
                            fin_a(n - 1)
                        elif item[0] == "finb":
                            fin_b(n - 1)
                        elif item[0] == "finc":
                            fin_c(n - 1)
                        else:
                            fin_d(n - 1)
                emit_exp(n, p, ps)
        for p in range(16 - LAG, 16):
            emit_attnv(3, p)
        fin_a(3)
        fin_b(3)
        fin_c(3)
        fin_d(3)

    nc.compile()
    return nc


def host_prep(x, gamma, beta, Wq, bq, Wk, bk, Wv, bv, Wo, bo):
    """Build the 8 per-core input dicts."""
    f32 = lambda a: np.ascontiguousarray(np.asarray(a, np.float32))
    x = f32(x)
    gamma, beta = f32(gamma), f32(beta)
    Wq, Wk, Wv, Wo = f32(Wq), f32(Wk), f32(Wv), f32(Wo)
    bq, bk, bv, bo = f32(bq), f32(bk), f32(bv), f32(bo)

    wq_dup = np.ascontiguousarray(np.concatenate([Wq, Wq], axis=1))
    z = np.zeros((64, 64), np.float32)
    wk_blk = np.ascontiguousarray(np.block([[Wk, z], [z, Wk]]))
    wv_blk = np.ascontiguousarray(np.block([[Wv, z], [z, Wv]]))
    comb = np.zeros((128, 128), np.float32)
    comb[:64, :64] = 1.0 / 64.0
    comb[64:, 64:] = 1.0 / 64.0
    bo_f = bv @ Wo + bo  # fold v bias through the out-projection
    mdt_np = mybir.dt.np(MDT)
    m = lambda a: np.ascontiguousarray(a).astype(mdt_np)
    shared = {
        "wq": m(wq_dup), "wk": m(wk_blk), "wv": m(wv_blk), "wo": m(Wo),
        "bq": np.ascontiguousarray(np.tile(bq, 2)[:, None]),
        "bo": np.ascontiguousarray(
            np.concatenate([bo_f, np.zeros(64, np.float32)])[:, None]),
        "gam": np.ascontiguousarray(np.tile(gamma, 2)[:, None]),
        "bet": np.ascontiguousarray(np.tile(beta, 2)[:, None]),
        "comb": comb,
    }
    in_maps = []
    for core in range(8):
        b, h = core // 2, core % 2
        xT = x[b].reshape(HW, C).T  # [64, 4096]
        halves = xT.reshape(C, 2, HALF)[:, [h, 1 - h], :]
        xp = np.ascontiguousarray(halves.transpose(1, 0, 2).reshape(128, HALF))
        in_maps.append({"xp": xp, **shared})
    return in_maps


def assemble(results, dtype):
    out = np.empty((B, HW, C), np.float32)
    for core in range(8):
        b, h = core // 2, core % 2
        out[b, HALF * h: HALF * h + HALF] = results[core]["out"].T
    return out.reshape(B, H, W, C).astype(dtype, copy=False)


_NC_CACHE = []


def kernel(x, gamma, beta, Wq, bq, Wk, bk, Wv, bv, Wo, bo):
    from concourse.bass_utils import run_bass_kernel_spmd

    if not _NC_CACHE:
        _NC_CACHE.append(build_nc())
    nc = _NC_CACHE[0]
    in_maps = host_prep(x, gamma, beta, Wq, bq, Wk, bk, Wv, bv, Wo, bo)
    res = run_bass_kernel_spmd(nc, in_maps, core_ids=list(range(8)))
    return assemble(res.results, np.asarray(x).dtype)


if __name__ == "__main__":
    rng = np.random.default_rng(0)
    inputs = {
        "x": rng.standard_normal((B, H, W, C)).astype(np.float32),
        "gamma": np.ones(C, np.float32), "beta": np.zeros(C, np.float32),
        "Wq": (rng.standard_normal((C, C)) / 8).astype(np.float32),
        "bq": np.zeros(C, np.float32),
        "Wk": (rng.standard_normal((C, C)) / 8).astype(np.float32),
        "bk": np.zeros(C, np.float32),
        "Wv": (rng.standard_normal((C, C)) / 8).astype(np.float32),
        "bv": np.zeros(C, np.float32),
        "Wo": (rng.standard_normal((C, C)) / 8).astype(np.float32),
        "bo": np.zeros(C, np.float32),
    }
    out = kernel(**inputs)
    print("kernel ran, out shape", out.shape, out.dtype)


# revision 37
# speedup vs baseline: 1.3859x; 1.0142x over previous
"""Trainium2 Bass kernel for nn_AttentionBlock (B=4, H=W=64, C=64, GroupNorm(8) +
full spatial self-attention), distributed over 8 NeuronCores.

Sharding: core i handles batch b=i//2 and query-half h=i%2 (2048 of the 4096
spatial positions). Each core computes the full GroupNorm and K/V for its
image (cheap) and attention only for its query half. No collectives.

v2 pipeline:
- exp split across THREE engines: ACT (table exp) for some score pairs,
  Pool (gpsimd) and DVE for the rest via a single-op int16 Schraudolph
  (i16 = s*23.083 + 16256.5 truncated, bitcast bf16 ~= e^(s/8), max rel err
  ~4%, final output err ~3e-3; denominator uses the same approximated
  weights so softmax normalization stays consistent).
- PE stream is gap-free: warmup matmuls ramp the clock during GroupNorm
  stats, then per tile scores-pair p / attnV pair p-3 alternate, with
  qk/v production and prev-tile finish matmuls slotted into the bubbles.
- biases: bq/bk folded into the q/k PSUM->SBUF copies (per-partition add);
  bv folded into bo on the host (bv @ Wo + bo).
- softmax denominators ride as a 65th ones-column of V; reciprocal via
  the fast custom-DVE op on [1,512] then broadcast by a bf16 PE matmul.
"""

import sys

sys.path.insert(0, "/opt/trn_rl_repo")

import numpy as np

import concourse.bacc as bacc
import concourse.tile as tile
from concourse import mybir

B, H, W, C = 4, 64, 64, 64
HW = H * W  # 4096
HALF = HW // 2  # 2048
EPS = 1e-5
SCALE = C ** -0.5

F32 = mybir.dt.float32
MDT = mybir.dt.bfloat16  # PE matmul operand dtype
I16 = mybir.dt.int16

# Schraudolph exp in bf16-bit space: i16 = round(s * 2^7/ln2 * SCALE + 127*2^7)
SCH_SCALE = float((2.0 ** 7) / np.log(2.0) * SCALE)
SCH_BIAS = 16251.0  # 127*2^7 shifted -5.5 to center the one-sided
# mantissa-interpolation error (+0..6.7%) around zero

NWARM = 24  # PE warmup matmuls (ramp p-state during GN stats)
LAGS = [6, 6, 6, 3]  # attnV trails scores by LAG pairs; short last tile
# so the post-loop drain is small

# engine per exp pair: A=ACT table exp, D=DVE int16-schraudolph. (Pool cannot
# read PSUM on TRN2, so it only gets SBUF->SBUF work: xn, recb, final out.)
# Tile tails lean A so DVE is clear for the next tile's start.
EMAP0 = ['D', 'A', 'D', 'A', 'D', 'A', 'D', 'A',
         'D', 'A', 'D', 'A', 'A', 'D', 'A', 'A']      # A9 D7
EMAPN = ['D', 'A', 'D', 'A', 'D', 'A', 'D', 'A',
         'D', 'D', 'A', 'D', 'D', 'A', 'A', 'A']      # A8 D8


def build_nc():
    nc = bacc.Bacc("TRN2", debug=False, num_devices=8)

    # ---- DRAM I/O ----
    xp_d = nc.dram_tensor("xp", [128, HALF], F32, kind="ExternalInput")
    wq_d = nc.dram_tensor("wq", [64, 128], MDT, kind="ExternalInput")
    wk_d = nc.dram_tensor("wk", [128, 128], MDT, kind="ExternalInput")
    wv_d = nc.dram_tensor("wv", [128, 128], MDT, kind="ExternalInput")
    wo_d = nc.dram_tensor("wo", [64, 64], MDT, kind="ExternalInput")
    bq_d = nc.dram_tensor("bq", [128, 1], F32, kind="ExternalInput")
    bo_d = nc.dram_tensor("bo", [128, 1], F32, kind="ExternalInput")
    gam_d = nc.dram_tensor("gam", [128, 1], F32, kind="ExternalInput")
    bet_d = nc.dram_tensor("bet", [128, 1], F32, kind="ExternalInput")
    comb_d = nc.dram_tensor("comb", [128, 128], F32, kind="ExternalInput")
    out_d = nc.dram_tensor("out", [64, HALF], F32, kind="ExternalOutput")

    with tile.TileContext(nc) as tc, \
         tc.tile_pool(name="singles", bufs=1) as singles, \
         tc.tile_pool(name="stats", bufs=1) as stats, \
         tc.tile_pool(name="sc_ps", bufs=2, space="PSUM") as sc_ps, \
         tc.tile_pool(name="pacc_ps", bufs=2, space="PSUM") as pacc_ps, \
         tc.tile_pool(name="aux_ps", bufs=1, space="PSUM") as aux_ps, \
         tc.tile_pool(name="work", bufs=2) as work:

        # ---- input DMAs: x alternates across the two hardware queues so the
        # GN stats chain starts sooner; weights follow on the ACT queue ----
        x_sb = singles.tile([128, HALF], F32)
        for r in range(4):
            eng = nc.sync if r % 2 == 0 else nc.scalar
            eng.dma_start(
                x_sb[:, 512 * r: 512 * r + 512],
                xp_d.ap()[:, 512 * r: 512 * r + 512],
            )
        gam_sb = singles.tile([128, 1], F32)
        nc.scalar.dma_start(gam_sb[:], gam_d.ap())
        bet_sb = singles.tile([128, 1], F32)
        nc.scalar.dma_start(bet_sb[:], bet_d.ap())
        comb_sb = singles.tile([128, 128], F32)
        nc.scalar.dma_start(comb_sb[:], comb_d.ap())
        wq_sb = singles.tile([64, 128], MDT)
        nc.scalar.dma_start(wq_sb[:], wq_d.ap())
        wk_sb = singles.tile([128, 128], MDT)
        nc.scalar.dma_start(wk_sb[:], wk_d.ap())
        bq_sb = singles.tile([128, 1], F32)
        nc.scalar.dma_start(bq_sb[:], bq_d.ap())
        wv_sb = singles.tile([128, 128], MDT)
        nc.scalar.dma_start(wv_sb[:], wv_d.ap())
        wo_sb = singles.tile([64, 64], MDT)
        nc.scalar.dma_start(wo_sb[:], wo_d.ap())
        bo_sb = singles.tile([128, 1], F32)
        nc.scalar.dma_start(bo_sb[:], bo_d.ap())

        # ---- big SBUF tensors ----
        xn_r = singles.tile([128, HALF], MDT)
        q_dup = singles.tile([128, HALF], MDT)
        kt_sb = singles.tile([128, HALF], MDT)
        v_all = singles.tile([128, 65 * 32], MDT)
        attnexp = singles.tile([128, 1024 * 16], MDT)
        out_sb = singles.tile([64, HALF], F32)
        ones_sb = singles.tile([128, 512], MDT)

        # constants on Pool, first thing (warmup matmuls read ones_sb)
        nc.gpsimd.memset(ones_sb[:], 1.0)
        v4 = v_all[:].rearrange("p (h t e) -> p h t e", h=2, e=65)
        nc.gpsimd.memset(v4[:, :, :, 64:65], 1.0)

        # pre-warm the sqrt ACT table set (used by the GN rstd); the exp
        # table is loaded right after the single sqrt below
        scr = stats.tile([128, 1], F32)
        nc.vector.memset(scr[:], 1.0)
        nc.scalar.activation(scr[:], scr[:], mybir.ActivationFunctionType.Sqrt)

        # ---- PE warmup: ramp the activity monitor while DVE does GN stats
        # (rides the sc-tag banks, which are free until the first scores) ----
        for w in range(NWARM):
            wps = sc_ps.tile([128, 512], F32, tag="sc", name=f"warm{w}")
            nc.tensor.matmul(wps[:], ones_sb[:, 0:128], ones_sb[:, :],
                             start=True, stop=True)

        # ---- GroupNorm stats: bn per partition per 512-slice, then a
        # block-diagonal averaging matmul combines across channels ----
        st6 = stats.tile([128, 4, 6], F32)
        mv4 = stats.tile([128, 4, 2], F32)
        for r in range(4):
            nc.vector.bn_stats(st6[:, r, :], x_sb[:, 512 * r: 512 * r + 512])
            nc.vector.bn_aggr(mv4[:, r, :], st6[:, r, :])
        smat = stats.tile([128, 8], F32)  # cols 0-3 mean, 4-7 E[x^2]
        nc.vector.tensor_copy(smat[:, 0:4], mv4[:, :, 0])
        nc.vector.tensor_mul(smat[:, 4:8], mv4[:, :, 0], mv4[:, :, 0])
        nc.vector.tensor_add(smat[:, 4:8], smat[:, 4:8], mv4[:, :, 1])

        cps = pacc_ps.tile([128, 8], F32, tag="pacc")
        nc.tensor.matmul(cps[:], comb_sb[:], smat[:], start=True, stop=True)
        gstat = stats.tile([128, 8], F32)  # 0-3 mean_g, 4-7 E2_g
        nc.vector.tensor_copy(gstat[:], cps[:])

        # var = E2 - mean^2; EPS folds into the sqrt's activation bias.
        # rstd = 1/sqrt(var+EPS) via ACT sqrt + fast custom-DVE reciprocal
        # (the sanctioned accurate path; far fewer serial ops than a
        # bit-trick Newton chain)
        ve = stats.tile([128, 4], F32)
        nc.vector.tensor_mul(ve[:], gstat[:, 0:4], gstat[:, 0:4])
        nc.vector.tensor_sub(ve[:], gstat[:, 4:8], ve[:])
        eps_sb = stats.tile([128, 1], F32)
        nc.vector.memset(eps_sb[:], EPS)
        sve = stats.tile([128, 4], F32)
        nc.scalar.activation(sve[:], ve[:],
                             mybir.ActivationFunctionType.Sqrt,
                             bias=eps_sb[:])
        # switch ACT to the exp table now, during idle time
        nc.scalar.activation(scr[:], scr[:], mybir.ActivationFunctionType.Exp)
        rstd = stats.tile([128, 4], F32)
        nc.vector.reciprocal_approx_fast(out=rstd[:], in_=sve[:])

        gsc = stats.tile([128, 4], F32)
        nc.vector.tensor_scalar_mul(gsc[:], rstd[:], gam_sb[:])
        gbias = stats.tile([128, 4], F32)
        nc.vector.tensor_mul(gbias[:], gstat[:, 0:4], gsc[:])
        nc.vector.tensor_scalar(
            out=gbias[:], in0=gbias[:], scalar1=-1.0, scalar2=bet_sb[:],
            op0=mybir.AluOpType.mult, op1=mybir.AluOpType.add,
        )
        # the fp32 residual pass folds in bo (bo rides rows 0:63 of the bias;
        # rows 64:127 of x_sb are never read again after the qkv matmuls)
        gbias2 = stats.tile([128, 4], F32)
        nc.vector.tensor_scalar_add(gbias2[:], gbias[:], bo_sb[:])
        # xn = x * gsc + gbias: both the bf16 matmul copy and the fp32
        # residual pass run on Pool (SBUF->SBUF), keeping ACT/DVE free for
        # the PSUM-side attention work. Per slice: bf16 first, then the
        # in-place fp32 overwrite (same engine => ordered).
        for r in range(4):
            sl = slice(512 * r, 512 * r + 512)
            nc.gpsimd.tensor_scalar(
                out=xn_r[:, sl], in0=x_sb[:, sl],
                scalar1=gsc[:, r: r + 1], scalar2=gbias[:, r: r + 1],
                op0=mybir.AluOpType.mult, op1=mybir.AluOpType.add,
            )
            nc.gpsimd.tensor_scalar(
                out=x_sb[:, sl], in0=x_sb[:, sl],
                scalar1=gsc[:, r: r + 1], scalar2=gbias2[:, r: r + 1],
                op0=mybir.AluOpType.mult, op1=mybir.AluOpType.add,
            )

        # ---- emission helpers ----
        def emit_qk_slice(t, pool_tags):
            # k^T packed by half (lhsT = blockdiag(Wk, Wk)); q^T duplicated on
            # both partition halves (lhsT = [Wq | Wq]). bk is dropped exactly
            # (a per-query score constant cancels in softmax); bq folds into
            # the q copy as a per-partition bias on ACT.
            sl = slice(512 * t, 512 * t + 512)
            pool_k, tag_k = pool_tags[0]
            pool_q, tag_q = pool_tags[1]
            ps2 = pool_k.tile([128, 512], F32, tag=tag_k, name=f"kps{t}")
            nc.tensor.matmul(ps2[:], wk_sb[:], xn_r[:, sl], start=True,
                             stop=True)
            nc.vector.tensor_copy(kt_sb[:, sl], ps2[:])
            ps = pool_q.tile([128, 512], F32, tag=tag_q, name=f"qps{t}")
            nc.tensor.matmul(ps[:], wq_sb[:], xn_r[0:64, sl], start=True,
                             stop=True)
            nc.scalar.activation(
                q_dup[:, sl], ps[:], mybir.ActivationFunctionType.Identity,
                bias=bq_sb[:],
            )

        def emit_v(u, pool_tag=None):
            # v position-major, two 128-position chunks (halves) per matmul;
            # single copy with a dual-chunk strided output AP. Even u on the
            # bcq psum tag + DVE copy, odd u on fpq + ACT.
            sl = slice(128 * u, 128 * u + 128)
            pool, tag = pool_tag or (aux_ps, "bcq" if u % 2 == 0 else "fpq")
            ps = pool.tile([128, 128], F32, tag=tag, name=f"vps{u}")
            nc.tensor.matmul(ps[:], xn_r[:, sl], wv_sb[:], start=True,
                             stop=True)
            psr = ps[:].rearrange("p (h e) -> p h e", h=2)
            if u % 2 == 0:
                nc.vector.tensor_copy(v4[:, :, u, 0:64], psr[:, :, :])
            else:
                nc.scalar.activation(
                    v4[:, :, u, 0:64], psr[:, :, :],
                    mybir.ActivationFunctionType.Identity,
                )

        def emit_scores(n, p):
            # pair p: kv chunks p (half0, PE rows 0-63) and p+16 (half1, rows
            # 64-127) run concurrently; one [128,1024] 2-bank psum tile
            qsl = slice(512 * n, 512 * n + 512)
            ksl = slice(128 * p, 128 * p + 128)
            ps = sc_ps.tile([128, 1024], F32, tag="sc", name=f"sc{n}_{p}")
            nc.tensor.matmul(ps[:, 0:512], kt_sb[0:64, ksl],
                             q_dup[0:64, qsl], start=True, stop=True)
            nc.tensor.matmul(ps[:, 512:1024], kt_sb[64:128, ksl],
                             q_dup[64:128, qsl], start=True, stop=True)
            return ps

        def emit_exp(n, p, ps):
            # attnexp layout pair-major: chunk p at 1024p, chunk p+16 at
            # 1024p+512 — both written by this single instruction
            dst = attnexp[:, 1024 * p: 1024 * p + 1024]
            e = (EMAP0 if n == 0 else EMAPN)[p]
            if e == 'A':
                nc.scalar.activation(dst, ps[:],
                                     mybir.ActivationFunctionType.Exp,
                                     scale=SCALE)
            else:
                nc.vector.tensor_scalar(
                    out=dst.bitcast(I16), in0=ps[:],
                    scalar1=SCH_SCALE, scalar2=SCH_BIAS,
                    op0=mybir.AluOpType.mult, op1=mybir.AluOpType.add,
                )

        paccs = {}

        def emit_attnv(n, p):
            # kv chunk pair (p, p+16) — consumes exp pair p
            if n not in paccs:
                paccs[n] = pacc_ps.tile([65, 512], F32, tag="pacc",
                                        name=f"pacc{n}")
            pacc = paccs[n]
            for t in (p, p + 16):
                off = 1024 * p + (512 if t >= 16 else 0)
                nc.tensor.matmul(
                    pacc[:], v_all[:, 65 * t: 65 * t + 65],
                    attnexp[:, off: off + 512],
                    start=(t == 0), stop=(t == 31),
                )

        # finish chain for tile n, split into steps scheduled across pairs of
        # tile n+1 so the PE stream stays dense
        fin = {}

        def fin_a(n):
            # free the PSUM accumulator ASAP: unnormalized proj rows (bf16)
            # on ACT; the raw denominator row stays in PSUM for fin_b's rec
            pacc = paccs[n]
            projn_u = work.tile([64, 512], MDT, tag="projn", name=f"pn{n}")
            nc.scalar.activation(projn_u[:], pacc[0:64, :],
                                 mybir.ActivationFunctionType.Identity)
            fin[n] = (projn_u,)

        def fin_b(n):
            # per-query 1/denom straight off PSUM (fast custom-DVE approx).
            # The custom op ignores input partition offsets, so run it over
            # all 65 partitions (same cost: DVE time = free size) and use
            # row 64. Then a tiny bf16 convert on Pool for the broadcast.
            pacc = paccs.pop(n)
            (projn_u,) = fin[n]
            rec = work.tile([65, 512], F32, tag="rec", name=f"rec{n}")
            nc.vector.reciprocal_approx_fast(out=rec[:], in_=pacc[:, :])
            recb = work.tile([1, 512], MDT, tag="recb", name=f"recb{n}")
            nc.gpsimd.tensor_copy(recb[:], rec[64:65, :])
            fin[n] = (projn_u, recb)

        def fin_c(n):
            # PE: broadcast 1/denom to [64,512] + out-projection
            projn_u, recb = fin[n]
            bc_ps = aux_ps.tile([64, 512], F32, tag="bcq", name=f"bc{n}")
            nc.tensor.matmul(bc_ps[:], ones_sb[0:1, 0:64], recb[:],
                             start=True, stop=True)
            fps = aux_ps.tile([64, 512], F32, tag="fpq", name=f"fps{n}")
            nc.tensor.matmul(fps[:], wo_sb[:], projn_u[:], start=True,
                             stop=True)
            bc_sb = work.tile([64, 512], F32, tag="bc", name=f"bcs{n}")
            nc.scalar.activation(bc_sb[:], bc_ps[:],
                                 mybir.ActivationFunctionType.Identity)
            fin[n] = (fps, bc_sb)

        def fin_d(n):
            # normalize on DVE, then bias + residual + store on Pool
            fps, bc_sb = fin.pop(n)
            qsl = slice(512 * n, 512 * n + 512)
            mn = work.tile([64, 512], F32, tag="mn", name=f"mn{n}")
            nc.vector.tensor_mul(mn[:], fps[:], bc_sb[:])
            nc.gpsimd.tensor_add(out_sb[:, qsl], mn[:], x_sb[0:64, qsl])
            nc.sync.dma_start(out_d.ap()[:, qsl], out_sb[:, qsl])

        # ---- software-pipelined attention ----
        # tile 0 extras: qk slices 1-3 and v chunks produced just in time
        # (scores pair p needs kt slice p//4, attnV pair p-LAG needs v chunk
        # p-LAG). The earliest qkv psums ride the pacc-tag banks (free until
        # the first pacc allocation at p=LAG); the rest alternate bcq/fpq so
        # every tenant's copy has >= 2 pairs to drain before bank reuse.
        PACC_TAG = (pacc_ps, "pacc")
        T0_EXTRA = {0: [("qk", 1, (PACC_TAG, PACC_TAG))],
                    1: [("v", 0, PACC_TAG), ("v", 1, PACC_TAG)],
                    2: [("v", 2, None), ("v", 3, None)],
                    3: [("qk", 2, None)],
                    4: [("v", 4, None), ("v", 5, None)],
                    5: [("v", 6, None)], 6: [("v", 7, None)],
                    7: [("qk", 3, None)],
                    8: [("v", 8, None)], 9: [("v", 9, None)],
                    10: [("v", 10, None)], 11: [("v", 11, None)],
                    12: [("v", 12, None)], 13: [("v", 13, None)],
                    14: [("v", 14, None)], 15: [("v", 15, None)]}
        # tiles 1-3: previous tile's spill attnV pairs + finish steps (spread
        # out so each step's engine work has slack before its consumer)
        TN_EXTRA = {0: [("spill", 10)], 1: [("spill", 11)],
                    2: [("spill", 12)], 3: [("spill", 13)],
                    4: [("spill", 14)], 5: [("spill", 15)],
                    6: [("fina",)], 7: [("finb",)],
                    9: [("finc",)], 11: [("find",)]}
        AUX = ((aux_ps, "bcq"), (aux_ps, "fpq"))

        emit_qk_slice(0, AUX)
        for n in range(4):
            for p in range(16):
                ps = emit_scores(n, p)
                if p >= LAGS[n]:
                    emit_attnv(n, p - LAGS[n])
                if n == 0:
                    for item in T0_EXTRA.get(p, []):
                        if item[0] == "qk":
                            emit_qk_slice(item[1], item[2] or AUX)
                        else:
                            emit_v(item[1], item[2])
                else:
                    for item in TN_EXTRA.get(p, []):
                        if item[0] == "spill":
                            emit_attnv(n - 1, item[1])
                        elif item[0] == "fina":
                            fin_a(n - 1)
                        elif item[0] == "finb":
                            fin_b(n - 1)
                        elif item[0] == "finc":
                            fin_c(n - 1)
                        else:
                            fin_d(n - 1)
                emit_exp(n, p, ps)
        for p in range(16 - LAGS[3], 16):
            emit_attnv(3, p)
        fin_a(3)
        fin_b(3)
        fin_c(3)
        fin_d(3)

    nc.compile()
    return nc


def host_prep(x, gamma, beta, Wq, bq, Wk, bk, Wv, bv, Wo, bo):
    """Build the 8 per-core input dicts."""
    f32 = lambda a: np.ascontiguousarray(np.asarray(a, np.float32))
    x = f32(x)
    gamma, beta = f32(gamma), f32(beta)
    Wq, Wk, Wv, Wo = f32(Wq), f32(Wk), f32(Wv), f32(Wo)
    bq, bk, bv, bo = f32(bq), f32(bk), f32(bv), f32(bo)

    wq_dup = np.ascontiguousarray(np.concatenate([Wq, Wq], axis=1))
    z = np.zeros((64, 64), np.float32)
    wk_blk = np.ascontiguousarray(np.block([[Wk, z], [z, Wk]]))
    wv_blk = np.ascontiguousarray(np.block([[Wv, z], [z, Wv]]))
    comb = np.zeros((128, 128), np.float32)
    comb[:64, :64] = 1.0 / 64.0
    comb[64:, 64:] = 1.0 / 64.0
    bo_f = bv @ Wo + bo  # fold v bias through the out-projection
    mdt_np = mybir.dt.np(MDT)
    m = lambda a: np.ascontiguousarray(a).astype(mdt_np)
    shared = {
        "wq": m(wq_dup), "wk": m(wk_blk), "wv": m(wv_blk), "wo": m(Wo),
        "bq": np.ascontiguousarray(np.tile(bq, 2)[:, None]),
        "bo": np.ascontiguousarray(
            np.concatenate([bo_f, np.zeros(64, np.float32)])[:, None]),
        "gam": np.ascontiguousarray(np.tile(gamma, 2)[:, None]),
        "bet": np.ascontiguousarray(np.tile(beta, 2)[:, None]),
        "comb": comb,
    }
    in_maps = []
    for core in range(8):
        b, h = core // 2, core % 2
        xT = x[b].reshape(HW, C).T  # [64, 4096]
        halves = xT.reshape(C, 2, HALF)[:, [h, 1 - h], :]
        xp = np.ascontiguousarray(halves.transpose(1, 0, 2).reshape(128, HALF))
        in_maps.append({"xp": xp, **shared})
    return in_maps


def assemble(results, dtype):
    out = np.empty((B, HW, C), np.float32)
    for core in range(8):
        b, h = core // 2, core % 2
        out[b, HALF * h: HALF * h + HALF] = results[core]["out"].T
    return out.reshape(B, H, W, C).astype(dtype, copy=False)


_NC_CACHE = []


def kernel(x, gamma, beta, Wq, bq, Wk, bk, Wv, bv, Wo, bo):
    from concourse.bass_utils import run_bass_kernel_spmd

    if not _NC_CACHE:
        _NC_CACHE.append(build_nc())
    nc = _NC_CACHE[0]
    in_maps = host_prep(x, gamma, beta, Wq, bq, Wk, bk, Wv, bv, Wo, bo)
    res = run_bass_kernel_spmd(nc, in_maps, core_ids=list(range(8)))
    return assemble(res.results, np.asarray(x).dtype)


if __name__ == "__main__":
    rng = np.random.default_rng(0)
    inputs = {
        "x": rng.standard_normal((B, H, W, C)).astype(np.float32),
        "gamma": np.ones(C, np.float32), "beta": np.zeros(C, np.float32),
        "Wq": (rng.standard_normal((C, C)) / 8).astype(np.float32),
        "bq": np.zeros(C, np.float32),
        "Wk": (rng.standard_normal((C, C)) / 8).astype(np.float32),
        "bk": np.zeros(C, np.float32),
        "Wv": (rng.standard_normal((C, C)) / 8).astype(np.float32),
        "bv": np.zeros(C, np.float32),
        "Wo": (rng.standard_normal((C, C)) / 8).astype(np.float32),
        "bo": np.zeros(C, np.float32),
    }
    out = kernel(**inputs)
    print("kernel ran, out shape", out.shape, out.dtype)


# revision 44
# speedup vs baseline: 1.5292x; 1.1034x over previous
"""Trainium2 Bass kernel for nn_AttentionBlock (B=4, H=W=64, C=64, GroupNorm(8) +
full spatial self-attention), distributed over 8 NeuronCores.

Sharding: core i handles batch b=i//2 and query-half h=i%2 (2048 of the 4096
spatial positions). Each core computes the full GroupNorm and K/V for its
image (cheap) and attention only for its query half. No collectives.

v2 pipeline:
- exp split across THREE engines: ACT (table exp) for some score pairs,
  Pool (gpsimd) and DVE for the rest via a single-op int16 Schraudolph
  (i16 = s*23.083 + 16256.5 truncated, bitcast bf16 ~= e^(s/8), max rel err
  ~4%, final output err ~3e-3; denominator uses the same approximated
  weights so softmax normalization stays consistent).
- PE stream is gap-free: warmup matmuls ramp the clock during GroupNorm
  stats, then per tile scores-pair p / attnV pair p-3 alternate, with
  qk/v production and prev-tile finish matmuls slotted into the bubbles.
- biases: bq/bk folded into the q/k PSUM->SBUF copies (per-partition add);
  bv folded into bo on the host (bv @ Wo + bo).
- softmax denominators ride as a 65th ones-column of V; reciprocal via
  the fast custom-DVE op on [1,512] then broadcast by a bf16 PE matmul.
"""

import sys

sys.path.insert(0, "/opt/trn_rl_repo")

import numpy as np

import concourse.bacc as bacc
import concourse.tile as tile
from concourse import mybir

B, H, W, C = 4, 64, 64, 64
HW = H * W  # 4096
HALF = HW // 2  # 2048
EPS = 1e-5
SCALE = C ** -0.5

F32 = mybir.dt.float32
MDT = mybir.dt.bfloat16  # PE matmul operand dtype
I16 = mybir.dt.int16

# Schraudolph exp in bf16-bit space: i16 = round(s * 2^7/ln2 * SCALE + 127*2^7)
SCH_SCALE = float((2.0 ** 7) / np.log(2.0) * SCALE)
SCH_BIAS = 16251.0  # 127*2^7 shifted -5.5 to center the one-sided
# mantissa-interpolation error (+0..6.7%) around zero

NWARM = 24  # PE warmup matmuls (ramp p-state during GN stats)
LAGS = [6, 6, 6, 3]  # attnV trails scores by LAG pairs; short last tile
# so the post-loop drain is small

# engine per exp pair: A=ACT table exp, D=DVE int16-schraudolph. (Pool cannot
# read PSUM on TRN2, so it only gets SBUF->SBUF work: xn, recb, final out.)
# Tile tails lean A so DVE is clear for the next tile's start.
EMAP0 = ['D', 'A', 'D', 'A', 'D', 'A', 'D', 'A',
         'D', 'A', 'D', 'A', 'A', 'D', 'A', 'A']      # A9 D7
EMAPN = ['D', 'A', 'D', 'A', 'D', 'A', 'D', 'A',
         'D', 'D', 'A', 'D', 'D', 'A', 'A', 'A']      # A8 D8
EMAP3 = ['D', 'A'] * 8  # strict alternation: tile 3 runs LAG=3, so each
# exp must land within ~2 pairs of its scores
EMAPS = [EMAP0, EMAPN, EMAPN, EMAP3]


def build_nc():
    nc = bacc.Bacc("TRN2", debug=False, num_devices=8)

    # ---- DRAM I/O ----
    xp_d = nc.dram_tensor("xp", [128, HALF], F32, kind="ExternalInput")
    wq_d = nc.dram_tensor("wq", [64, 128], MDT, kind="ExternalInput")
    wk_d = nc.dram_tensor("wk", [128, 128], MDT, kind="ExternalInput")
    wv_d = nc.dram_tensor("wv", [128, 128], MDT, kind="ExternalInput")
    wo_d = nc.dram_tensor("wo", [64, 64], MDT, kind="ExternalInput")
    bq_d = nc.dram_tensor("bq", [128, 1], F32, kind="ExternalInput")
    bo_d = nc.dram_tensor("bo", [128, 1], F32, kind="ExternalInput")
    gam_d = nc.dram_tensor("gam", [128, 1], F32, kind="ExternalInput")
    bet_d = nc.dram_tensor("bet", [128, 1], F32, kind="ExternalInput")
    comb_d = nc.dram_tensor("comb", [128, 128], F32, kind="ExternalInput")
    out_d = nc.dram_tensor("out", [64, HALF], F32, kind="ExternalOutput")

    with tile.TileContext(nc) as tc, \
         tc.tile_pool(name="singles", bufs=1) as singles, \
         tc.tile_pool(name="stats", bufs=1) as stats, \
         tc.tile_pool(name="sc_ps", bufs=2, space="PSUM") as sc_ps, \
         tc.tile_pool(name="pacc_ps", bufs=2, space="PSUM") as pacc_ps, \
         tc.tile_pool(name="aux_ps", bufs=1, space="PSUM") as aux_ps, \
         tc.tile_pool(name="work", bufs=2) as work:

        # ---- input DMAs: everything on the sync hwdge queue (the issuing
        # engine is otherwise idle; putting DMAs on the ACT queue would
        # block the sqrt/exp table work behind descriptor writes) ----
        x_sb = singles.tile([128, HALF], F32)
        for r in range(4):
            nc.sync.dma_start(
                x_sb[:, 512 * r: 512 * r + 512],
                xp_d.ap()[:, 512 * r: 512 * r + 512],
            )
        gam_sb = singles.tile([128, 1], F32)
        nc.sync.dma_start(gam_sb[:], gam_d.ap())
        bet_sb = singles.tile([128, 1], F32)
        nc.sync.dma_start(bet_sb[:], bet_d.ap())
        comb_sb = singles.tile([128, 128], F32)
        nc.sync.dma_start(comb_sb[:], comb_d.ap())
        wk_sb = singles.tile([128, 128], MDT)
        nc.sync.dma_start(wk_sb[:], wk_d.ap())
        wq_sb = singles.tile([64, 128], MDT)
        nc.sync.dma_start(wq_sb[:], wq_d.ap())
        bq_sb = singles.tile([128, 1], F32)
        nc.sync.dma_start(bq_sb[:], bq_d.ap())
        wv_sb = singles.tile([128, 128], MDT)
        nc.sync.dma_start(wv_sb[:], wv_d.ap())
        wo_sb = singles.tile([64, 64], MDT)
        nc.sync.dma_start(wo_sb[:], wo_d.ap())
        bo_sb = singles.tile([128, 1], F32)
        nc.sync.dma_start(bo_sb[:], bo_d.ap())

        # ---- big SBUF tensors ----
        xn_r = singles.tile([128, HALF], MDT)
        q_dup = singles.tile([128, HALF], MDT)
        kt_sb = singles.tile([128, HALF], MDT)
        v_all = singles.tile([128, 65 * 32], MDT)
        attnexp = singles.tile([128, 1024 * 16], MDT)
        out_sb = singles.tile([64, HALF], F32)
        ones_sb = singles.tile([128, 512], MDT)

        # constants on Pool, first thing (warmup matmuls read ones_sb)
        nc.gpsimd.memset(ones_sb[:], 1.0)
        v4 = v_all[:].rearrange("p (h t e) -> p h t e", h=2, e=65)
        nc.gpsimd.memset(v4[:, :, :, 64:65], 1.0)

        # pre-warm the sqrt ACT table set (used by the GN rstd); the exp
        # table is loaded right after the single sqrt below
        scr = stats.tile([128, 1], F32)
        nc.vector.memset(scr[:], 1.0)
        nc.scalar.activation(scr[:], scr[:], mybir.ActivationFunctionType.Sqrt)

        # ---- PE warmup: ramp the activity monitor while DVE does GN stats
        # (rides the sc-tag banks, which are free until the first scores) ----
        for w in range(NWARM):
            wps = sc_ps.tile([128, 512], F32, tag="sc", name=f"warm{w}")
            nc.tensor.matmul(wps[:], ones_sb[:, 0:128], ones_sb[:, :],
                             start=True, stop=True)

        # ---- GroupNorm stats: bn per partition per 512-slice, then a
        # block-diagonal averaging matmul combines across channels ----
        st6 = stats.tile([128, 4, 6], F32)
        mv4 = stats.tile([128, 4, 2], F32)
        for r in range(4):
            nc.vector.bn_stats(st6[:, r, :], x_sb[:, 512 * r: 512 * r + 512])
            nc.vector.bn_aggr(mv4[:, r, :], st6[:, r, :])
        smat = stats.tile([128, 8], F32)  # cols 0-3 mean, 4-7 E[x^2]
        nc.vector.tensor_copy(smat[:, 0:4], mv4[:, :, 0])
        nc.vector.tensor_mul(smat[:, 4:8], mv4[:, :, 0], mv4[:, :, 0])
        nc.vector.tensor_add(smat[:, 4:8], smat[:, 4:8], mv4[:, :, 1])

        cps = pacc_ps.tile([128, 8], F32, tag="pacc")
        nc.tensor.matmul(cps[:], comb_sb[:], smat[:], start=True, stop=True)
        gstat = stats.tile([128, 8], F32)  # 0-3 mean_g, 4-7 E2_g
        nc.vector.tensor_copy(gstat[:], cps[:])

        # var = E2 - mean^2; EPS folds into the sqrt's activation bias.
        # rstd = 1/sqrt(var+EPS) via ACT sqrt + fast custom-DVE reciprocal
        # (the sanctioned accurate path; far fewer serial ops than a
        # bit-trick Newton chain)
        ve = stats.tile([128, 4], F32)
        nc.vector.tensor_mul(ve[:], gstat[:, 0:4], gstat[:, 0:4])
        nc.vector.tensor_sub(ve[:], gstat[:, 4:8], ve[:])
        eps_sb = stats.tile([128, 1], F32)
        nc.vector.memset(eps_sb[:], EPS)
        sve = stats.tile([128, 4], F32)
        nc.scalar.activation(sve[:], ve[:],
                             mybir.ActivationFunctionType.Sqrt,
                             bias=eps_sb[:])
        # switch ACT to the exp table now, during idle time
        nc.scalar.activation(scr[:], scr[:], mybir.ActivationFunctionType.Exp)
        rstd = stats.tile([128, 4], F32)
        nc.vector.reciprocal_approx_fast(out=rstd[:], in_=sve[:])

        gsc = stats.tile([128, 4], F32)
        nc.vector.tensor_scalar_mul(gsc[:], rstd[:], gam_sb[:])
        gbias = stats.tile([128, 4], F32)
        nc.vector.tensor_mul(gbias[:], gstat[:, 0:4], gsc[:])
        nc.vector.tensor_scalar(
            out=gbias[:], in0=gbias[:], scalar1=-1.0, scalar2=bet_sb[:],
            op0=mybir.AluOpType.mult, op1=mybir.AluOpType.add,
        )
        # the fp32 residual pass folds in bo (bo rides rows 0:63 of the bias;
        # rows 64:127 of x_sb are never read again after the qkv matmuls)
        gbias2 = stats.tile([128, 4], F32)
        nc.vector.tensor_scalar_add(gbias2[:], gbias[:], bo_sb[:])
        # xn = x * gsc + gbias: slice 0's bf16 copy runs on DVE (it gates
        # qk0 -> first scores); the rest and the fp32 residual pass run on
        # the slow-but-idle Pool. Per slice: bf16 read first, then the
        # in-place fp32 overwrite (Pool ops ordered; DVE xn0 emitted before
        # Pool's slice-0 overwrite so the framework serializes the WAR).
        nc.vector.tensor_scalar(
            out=xn_r[:, 0:512], in0=x_sb[:, 0:512],
            scalar1=gsc[:, 0:1], scalar2=gbias[:, 0:1],
            op0=mybir.AluOpType.mult, op1=mybir.AluOpType.add,
        )
        for r in range(4):
            sl = slice(512 * r, 512 * r + 512)
            if r > 0:
                nc.gpsimd.tensor_scalar(
                    out=xn_r[:, sl], in0=x_sb[:, sl],
                    scalar1=gsc[:, r: r + 1], scalar2=gbias[:, r: r + 1],
                    op0=mybir.AluOpType.mult, op1=mybir.AluOpType.add,
                )
            nc.gpsimd.tensor_scalar(
                out=x_sb[:, sl], in0=x_sb[:, sl],
                scalar1=gsc[:, r: r + 1], scalar2=gbias2[:, r: r + 1],
                op0=mybir.AluOpType.mult, op1=mybir.AluOpType.add,
            )

        # ---- emission helpers ----
        def emit_qk_slice(t, pool_tags):
            # k^T packed by half (lhsT = blockdiag(Wk, Wk)); q^T duplicated on
            # both partition halves (lhsT = [Wq | Wq]). bk is dropped exactly
            # (a per-query score constant cancels in softmax); bq folds into
            # the q copy as a per-partition bias on ACT.
            sl = slice(512 * t, 512 * t + 512)
            pool_k, tag_k = pool_tags[0]
            pool_q, tag_q = pool_tags[1]
            ps2 = pool_k.tile([128, 512], F32, tag=tag_k, name=f"kps{t}")
            nc.tensor.matmul(ps2[:], wk_sb[:], xn_r[:, sl], start=True,
                             stop=True)
            nc.vector.tensor_copy(kt_sb[:, sl], ps2[:])
            ps = pool_q.tile([128, 512], F32, tag=tag_q, name=f"qps{t}")
            nc.tensor.matmul(ps[:], wq_sb[:], xn_r[0:64, sl], start=True,
                             stop=True)
            nc.scalar.activation(
                q_dup[:, sl], ps[:], mybir.ActivationFunctionType.Identity,
                bias=bq_sb[:],
            )

        def emit_v(u, pool_tag=None):
            # v position-major, two 128-position chunks (halves) per matmul;
            # single copy with a dual-chunk strided output AP. Even u on the
            # bcq psum tag + DVE copy, odd u on fpq + ACT.
            sl = slice(128 * u, 128 * u + 128)
            pool, tag = pool_tag or (aux_ps, "bcq" if u % 2 == 0 else "fpq")
            ps = pool.tile([128, 128], F32, tag=tag, name=f"vps{u}")
            nc.tensor.matmul(ps[:], xn_r[:, sl], wv_sb[:], start=True,
                             stop=True)
            psr = ps[:].rearrange("p (h e) -> p h e", h=2)
            nc.vector.tensor_copy(v4[:, :, u, 0:64], psr[:, :, :])

        def emit_scores(n, p):
            # pair p: kv chunks p (half0, PE rows 0-63) and p+16 (half1, rows
            # 64-127) run concurrently; one [128,1024] 2-bank psum tile
            qsl = slice(512 * n, 512 * n + 512)
            ksl = slice(128 * p, 128 * p + 128)
            ps = sc_ps.tile([128, 1024], F32, tag="sc", name=f"sc{n}_{p}")
            nc.tensor.matmul(ps[:, 0:512], kt_sb[0:64, ksl],
                             q_dup[0:64, qsl], start=True, stop=True)
            nc.tensor.matmul(ps[:, 512:1024], kt_sb[64:128, ksl],
                             q_dup[64:128, qsl], start=True, stop=True)
            return ps

        def emit_exp(n, p, ps):
            # attnexp layout pair-major: chunk p at 1024p, chunk p+16 at
            # 1024p+512 — both written by this single instruction
            dst = attnexp[:, 1024 * p: 1024 * p + 1024]
            e = EMAPS[n][p]
            if e == 'A':
                nc.scalar.activation(dst, ps[:],
                                     mybir.ActivationFunctionType.Exp,
                                     scale=SCALE)
            else:
                nc.vector.tensor_scalar(
                    out=dst.bitcast(I16), in0=ps[:],
                    scalar1=SCH_SCALE, scalar2=SCH_BIAS,
                    op0=mybir.AluOpType.mult, op1=mybir.AluOpType.add,
                )

        paccs = {}

        def emit_attnv(n, p):
            # kv chunk pair (p, p+16) — consumes exp pair p
            if n not in paccs:
                paccs[n] = pacc_ps.tile([65, 512], F32, tag="pacc",
                                        name=f"pacc{n}")
            pacc = paccs[n]
            for t in (p, p + 16):
                off = 1024 * p + (512 if t >= 16 else 0)
                nc.tensor.matmul(
                    pacc[:], v_all[:, 65 * t: 65 * t + 65],
                    attnexp[:, off: off + 512],
                    start=(t == 0), stop=(t == 31),
                )

        # finish chain for tile n, split into steps scheduled across pairs of
        # tile n+1 so the PE stream stays dense
        fin = {}

        def fin_a(n):
            # free the PSUM accumulator ASAP: unnormalized proj rows (bf16)
            # on ACT; the raw denominator row stays in PSUM for fin_b's rec
            pacc = paccs[n]
            projn_u = work.tile([64, 512], MDT, tag="projn", name=f"pn{n}")
            nc.scalar.activation(projn_u[:], pacc[0:64, :],
                                 mybir.ActivationFunctionType.Identity)
            fin[n] = (projn_u,)

        def fin_b(n):
            # per-query 1/denom straight off PSUM (fast custom-DVE approx).
            # The custom op ignores input partition offsets, so run it over
            # all 65 partitions (same cost: DVE time = free size) and use
            # row 64. Then a tiny bf16 convert on Pool for the broadcast.
            pacc = paccs.pop(n)
            (projn_u,) = fin[n]
            rec = work.tile([65, 512], F32, tag="rec", name=f"rec{n}")
            nc.vector.reciprocal_approx_fast(out=rec[:], in_=pacc[:, :])
            recb = work.tile([1, 512], MDT, tag="recb", name=f"recb{n}")
            nc.vector.tensor_copy(recb[:], rec[64:65, :])
            fin[n] = (projn_u, recb)

        def fin_c(n):
            # PE: broadcast 1/denom to [64,512] + out-projection
            projn_u, recb = fin[n]
            bc_ps = aux_ps.tile([64, 512], F32, tag="bcq", name=f"bc{n}")
            nc.tensor.matmul(bc_ps[:], ones_sb[0:1, 0:64], recb[:],
                             start=True, stop=True)
            fps = aux_ps.tile([64, 512], F32, tag="fpq", name=f"fps{n}")
            nc.tensor.matmul(fps[:], wo_sb[:], projn_u[:], start=True,
                             stop=True)
            bc_sb = work.tile([64, 512], F32, tag="bc", name=f"bcs{n}")
            nc.scalar.activation(bc_sb[:], bc_ps[:],
                                 mybir.ActivationFunctionType.Identity)
            fin[n] = (fps, bc_sb)

        def fin_d(n):
            # normalize on DVE, then bias + residual + store on Pool
            fps, bc_sb = fin.pop(n)
            qsl = slice(512 * n, 512 * n + 512)
            mn = work.tile([64, 512], F32, tag="mn", name=f"mn{n}")
            nc.vector.tensor_mul(mn[:], fps[:], bc_sb[:])
            # last tile's residual-add on DVE: it is the serial kernel tail
            eng = nc.vector if n == 3 else nc.gpsimd
            eng.tensor_add(out_sb[:, qsl], mn[:], x_sb[0:64, qsl])
            nc.sync.dma_start(out_d.ap()[:, qsl], out_sb[:, qsl])

        # ---- software-pipelined attention ----
        # tile 0 extras: qk slices 1-3 and v chunks produced just in time
        # (scores pair p needs kt slice p//4, attnV pair p-LAG needs v chunk
        # p-LAG). The earliest qkv psums ride the pacc-tag banks (free until
        # the first pacc allocation at p=LAG); the rest alternate bcq/fpq so
        # every tenant's copy has >= 2 pairs to drain before bank reuse.
        PACC_TAG = (pacc_ps, "pacc")
        T0_EXTRA = {0: [("qk", 1, (PACC_TAG, PACC_TAG))],
                    1: [("v", 0, PACC_TAG), ("v", 1, PACC_TAG)],
                    2: [("v", 2, None), ("v", 3, None)],
                    3: [("qk", 2, None)],
                    4: [("v", 4, None), ("v", 5, None)],
                    5: [("v", 6, None)], 6: [("v", 7, None)],
                    7: [("qk", 3, None)],
                    8: [("v", 8, None)], 9: [("v", 9, None)],
                    10: [("v", 10, None)], 11: [("v", 11, None)],
                    12: [("v", 12, None)], 13: [("v", 13, None)],
                    14: [("v", 14, None)], 15: [("v", 15, None)]}
        # tiles 1-3: previous tile's spill attnV pairs + finish steps (spread
        # out so each step's engine work has slack before its consumer)
        TN_EXTRA = {0: [("spill", 10)], 1: [("spill", 11)],
                    2: [("spill", 12)], 3: [("spill", 13)],
                    4: [("spill", 14)], 5: [("spill", 15)],
                    6: [("fina",)], 7: [("finb",)],
                    9: [("finc",)], 11: [("find",)]}
        AUX = ((aux_ps, "bcq"), (aux_ps, "fpq"))

        emit_qk_slice(0, AUX)
        for n in range(4):
            for p in range(16):
                ps = emit_scores(n, p)
                if p >= LAGS[n]:
                    emit_attnv(n, p - LAGS[n])
                if n == 0:
                    for item in T0_EXTRA.get(p, []):
                        if item[0] == "qk":
                            emit_qk_slice(item[1], item[2] or AUX)
                        else:
                            emit_v(item[1], item[2])
                else:
                    for item in TN_EXTRA.get(p, []):
                        if item[0] == "spill":
                            emit_attnv(n - 1, item[1])
                        elif item[0] == "fina":
                            fin_a(n - 1)
                        elif item[0] == "finb":
                            fin_b(n - 1)
                        elif item[0] == "finc":
                            fin_c(n - 1)
                        else:
                            fin_d(n - 1)
                emit_exp(n, p, ps)
        for p in range(16 - LAGS[3], 16):
            emit_attnv(3, p)
        fin_a(3)
        fin_b(3)
        fin_c(3)
        fin_d(3)

    nc.compile()
    return nc


def host_prep(x, gamma, beta, Wq, bq, Wk, bk, Wv, bv, Wo, bo):
    """Build the 8 per-core input dicts."""
    f32 = lambda a: np.ascontiguousarray(np.asarray(a, np.float32))
    x = f32(x)
    gamma, beta = f32(gamma), f32(beta)
    Wq, Wk, Wv, Wo = f32(Wq), f32(Wk), f32(Wv), f32(Wo)
    bq, bk, bv, bo = f32(bq), f32(bk), f32(bv), f32(bo)

    wq_dup = np.ascontiguousarray(np.concatenate([Wq, Wq], axis=1))
    z = np.zeros((64, 64), np.float32)
    wk_blk = np.ascontiguousarray(np.block([[Wk, z], [z, Wk]]))
    wv_blk = np.ascontiguousarray(np.block([[Wv, z], [z, Wv]]))
    comb = np.zeros((128, 128), np.float32)
    comb[:64, :64] = 1.0 / 64.0
    comb[64:, 64:] = 1.0 / 64.0
    bo_f = bv @ Wo + bo  # fold v bias through the out-projection
    mdt_np = mybir.dt.np(MDT)
    m = lambda a: np.ascontiguousarray(a).astype(mdt_np)
    shared = {
        "wq": m(wq_dup), "wk": m(wk_blk), "wv": m(wv_blk), "wo": m(Wo),
        "bq": np.ascontiguousarray(np.tile(bq, 2)[:, None]),
        "bo": np.ascontiguousarray(
            np.concatenate([bo_f, np.zeros(64, np.float32)])[:, None]),
        "gam": np.ascontiguousarray(np.tile(gamma, 2)[:, None]),
        "bet": np.ascontiguousarray(np.tile(beta, 2)[:, None]),
        "comb": comb,
    }
    in_maps = []
    for core in range(8):
        b, h = core // 2, core % 2
        xT = x[b].reshape(HW, C).T  # [64, 4096]
        halves = xT.reshape(C, 2, HALF)[:, [h, 1 - h], :]
        xp = np.ascontiguousarray(halves.transpose(1, 0, 2).reshape(128, HALF))
        in_maps.append({"xp": xp, **shared})
    return in_maps


def assemble(results, dtype):
    out = np.empty((B, HW, C), np.float32)
    for core in range(8):
        b, h = core // 2, core % 2
        out[b, HALF * h: HALF * h + HALF] = results[core]["out"].T
    return out.reshape(B, H, W, C).astype(dtype, copy=False)


_NC_CACHE = []


def kernel(x, gamma, beta, Wq, bq, Wk, bk, Wv, bv, Wo, bo):
    from concourse.bass_utils import run_bass_kernel_spmd

    if not _NC_CACHE:
        _NC_CACHE.append(build_nc())
    nc = _NC_CACHE[0]
    in_maps = host_prep(x, gamma, beta, Wq, bq, Wk, bk, Wv, bv, Wo, bo)
    res = run_bass_kernel_spmd(nc, in_maps, core_ids=list(range(8)))
    return assemble(res.results, np.asarray(x).dtype)


if __name__ == "__main__":
    rng = np.random.default_rng(0)
    inputs = {
        "x": rng.standard_normal((B, H, W, C)).astype(np.float32),
        "gamma": np.ones(C, np.float32), "beta": np.zeros(C, np.float32),
        "Wq": (rng.standard_normal((C, C)) / 8).astype(np.float32),
        "bq": np.zeros(C, np.float32),
        "Wk": (rng.standard_normal((C, C)) / 8).astype(np.float32),
        "bk": np.zeros(C, np.float32),
        "Wv": (rng.standard_normal((C, C)) / 8).astype(np.float32),
        "bv": np.zeros(C, np.float32),
        "Wo": (rng.standard_normal((C, C)) / 8).astype(np.float32),
        "bo": np.zeros(C, np.float32),
    }
    out = kernel(**inputs)
    print("kernel ran, out shape", out.shape, out.dtype)
